# revision 24
# baseline (speedup 1.0000x reference)
"""Trainium2 Bass kernel for GQA sliding-window attention (8-core SPMD).

Problem: B=8, S=32, D=4096, H=32 Q-heads, KVH=8 KV-heads, HD=128,
sliding window 4096 with 4064 cached positions.

Sharding: tensor-parallel over heads. Core c owns Q heads 4c..4c+3 and KV
head c (one GQA group): Wq/Wk/Wv column-sharded, cache sharded by KV head,
x replicated. Attention runs in two head-pair passes; after each pass the
cores all-gather that pass's (bf16) attention outputs through the runtime
collective stream, then each core applies a column slice of Wo; the host
concatenates column slices.

Layout/numerics notes:
  - All inputs are host-packed partition-major so every DMA lands with
    per-partition-contiguous multi-KB descriptor runs.
  - x is fed transposed (xT) so Q/K projections produce Q^T/K^T directly
    in [head_dim, token] layout. V is projected with xT chunks as the
    stationary operand instead, yielding V_new directly in [token, hd]
    layout (no PE transposes).
  - Wq/Wk columns (and cached K's hd axis) are permuted so RoPE's
    interleaved (even,odd) pairs become contiguous halves. The permutation
    cancels in q.k. SCALE is folded into Wq.
  - RoPE runs as qT = q * [c;c] + swap(q) * [-s;s] where swap exchanges
    the real/imag partition halves via one PE matmul against a host-fed
    128x128 permutation matrix. Three wide DVE ops per head pair replace
    the old 6-op/head serial chain.
  - Softmax skips max-subtraction; normalization is deferred: exp tiles
    are accumulated chunk-wise on the Vector engine into a [128, 512] f32
    carry, one ones-matmul per pass reduces it across partitions, and
    1/sum is applied when copying attention outputs out of PSUM.
  - Scores for two cache chunks accumulate into one 2-bank PSUM tile so a
    single ACTIVATE exps 2 chunks (halves Act instruction overhead).
"""

import os
import sys
from contextlib import ExitStack

import numpy as np
import ml_dtypes

import concourse.bass as bass
import concourse.tile as tile
import concourse.mybir as mybir
from concourse import bacc
from concourse.bass_utils import run_bass_kernel_spmd
from concourse.masks import make_identity

BF16 = ml_dtypes.bfloat16

CORES = 8
B, S, D = 8, 32, 4096
H, KVH, HD = 32, 8, 128
SW = 4096
PREV = SW - S  # 4064
TOK = B * S  # 256
NH = H // KVH  # 4 Q heads per core
NHP = NH // 2  # head pairs per core
QCOLS = NH * HD  # 512 Q-projection columns per core
SCALE = float(HD) ** -0.5

# hd permutation: interleaved (r0,i0,r1,i1,...) -> (r..., i...)
_IDX = np.concatenate([np.arange(0, HD, 2), np.arange(1, HD, 2)])

# exec time of the last traced run (ns), set when KERNEL_TRACE=1
LAST_EXEC_NS = None
LAST_RES = None

_BUILD_CACHE = {}


def _install_ntff_hook():
    """Register the axon NTFF profiling hook (the agent image's antenv stub
    lacks axon_hooks). Only needed when tracing."""
    import types

    if "antenv.axon_hooks" in sys.modules:
        return
    try:
        from trn_agent_boot.trn_boot import _ntff_profile_via_ctypes

        hook = _ntff_profile_via_ctypes("/opt/axon/libaxon_pjrt.so")
    except Exception:
        hook = None
    mod = types.ModuleType("antenv.axon_hooks")
    mod._hook = hook
    mod.get_axon_ntff_profile_hook = lambda: mod._hook
    mod.set_axon_ntff_profile_hook = lambda h: setattr(mod, "_hook", h)
    sys.modules["antenv.axon_hooks"] = mod
    import antenv

    antenv.axon_hooks = mod


def build(cores=CORES, gather=None):
    gather = gather or os.environ.get("KERNEL_GATHER", "cc")
    assert gather == "cc"
    n_dc = D // 128  # 32 contraction chunks for QKV projections
    n_tc = (PREV + 127) // 128  # cache t-chunks (last short)
    tail = PREV - (n_tc - 1) * 128  # 96
    outc = D // cores  # Wo output columns per core
    n_xp = 4  # xt/wq DMA pieces
    xp = n_dc // n_xp
    n_tg = n_tc // 2  # 2-chunk score/exp groups

    dt = mybir.dt
    bf, f32 = dt.bfloat16, dt.float32
    EXP = mybir.ActivationFunctionType.Exp

    nc = bacc.Bacc("TRN2", target_bir_lowering=False, debug=False, num_devices=cores)

    xt_d = nc.dram_tensor("xt", [128, n_dc, TOK], bf, kind="ExternalInput")
    wq_d = nc.dram_tensor("wq", [128, n_dc, QCOLS], bf, kind="ExternalInput")
    wkv_d = nc.dram_tensor("wkv", [128, n_dc, 2 * HD], bf, kind="ExternalInput")
    kct_d = nc.dram_tensor("kct", [HD, PREV], bf, kind="ExternalInput")
    vc_d = nc.dram_tensor("vc", [128, n_tc, HD], bf, kind="ExternalInput")
    wo_d = nc.dram_tensor("wo", [128, H, outc], bf, kind="ExternalInput")
    cc2_d = nc.dram_tensor("cc2", [128, TOK], bf, kind="ExternalInput")
    ss2_d = nc.dram_tensor("ss2", [128, TOK], bf, kind="ExternalInput")
    swp_d = nc.dram_tensor("swp", [128, 128], bf, kind="ExternalInput")
    maskt_d = nc.dram_tensor("maskt", [S, TOK], f32, kind="ExternalInput")
    out_d = nc.dram_tensor("out", [TOK, outc], f32, kind="ExternalOutput")

    with tile.TileContext(nc) as tc, ExitStack() as ctx:
        const = ctx.enter_context(tc.tile_pool(name="const", bufs=1))

        xt_sb = const.tile([128, n_dc, TOK], bf)
        wq_sb = const.tile([128, n_dc, QCOLS], bf)
        wkv_sb = const.tile([128, n_dc, 2 * HD], bf)
        kct_sb = const.tile([128, PREV], bf)
        vc_sb = const.tile([128, n_tc, HD], bf)
        wo_sb = const.tile([128, H, outc], bf)
        cc2_sb = const.tile([128, TOK], bf)
        ss2_sb = const.tile([128, TOK], bf)
        swp_sb = const.tile([128, 128], bf)
        maskt_sb = const.tile([S, B, S], f32)
        ones_sb = const.tile([128, 1], bf)
        ident_sb = const.tile([128, 128], bf)
        qsb = [const.tile([128, 2, TOK], bf, tag=f"qsb{p}", name=f"qsb{p}") for p in range(NHP)]
        ksb = const.tile([128, TOK], bf)
        qT_sb = [const.tile([128, 2, TOK], bf, tag=f"qT{p}", name=f"qT{p}") for p in range(NHP)]
        kTn_sb = const.tile([128, TOK], bf)
        vn_sb = const.tile([S, B, HD], bf)
        attn_new = [const.tile([S, 2, B, S], bf, tag=f"an{p}", name=f"an{p}") for p in range(NHP)]
        sacc_sb = const.tile([128, 2, TOK], f32, name="sacc")
        saccb_sb = const.tile([128, 2, TOK], bf, name="saccb")
        recip_sb = [const.tile([1, 2 * TOK], f32, tag=f"rc{p}", name=f"rc{p}") for p in range(NHP)]
        recipb_sb = [const.tile([1, 2 * TOK], bf, tag=f"rcb{p}", name=f"rcb{p}") for p in range(NHP)]
        recip_bc = [const.tile([128, 2 * TOK], bf, tag=f"rb{p}", name=f"rb{p}") for p in range(NHP)]
        attnout = [const.tile([128, 2 * TOK], bf, tag=f"ao{p}", name=f"ao{p}") for p in range(NHP)]
        allx = [
            const.tile([128, cores, 2 * TOK], bf, tag=f"all{p}", name=f"all{p}")
            for p in range(NHP)
        ]
        out_sb = const.tile([128, 2, outc], f32, name="out_sb")
        warm_sb = const.tile([128, 512], bf, name="warm_sb")

        # ---- cross-core launch sync: fire the tiny AllGather as early as
        # possible so the collective-stream rendezvous (which absorbs core
        # launch skew) overlaps the input DMA + projection phase. ----
        dram = ctx.enter_context(tc.tile_pool(name="dram", bufs=1, space="DRAM"))
        ag_in = [dram.tile([128, 2 * TOK], bf, tag=f"agi{p}", name=f"agi{p}") for p in range(NHP)]
        ag_out = [
            dram.tile([128 * cores, 2 * TOK], bf, tag=f"ago{p}", name=f"ago{p}",
                      addr_space="Shared")
            for p in range(NHP)
        ]

        nc.vector.memset(warm_sb[:], 0.0)

        # ---- input DMAs ----
        # All projection-critical bytes ride the scalar queue (measured
        # ~2x faster than the sync queue), interleaved in consumption
        # order so the chunk loop is paced with no cross-queue contention.
        # The sync queue takes everything needed later.
        for i in range(n_xp):
            sl = slice(i * xp, (i + 1) * xp)
            nc.scalar.dma_start(out=xt_sb[:, sl, :], in_=xt_d.ap()[:, sl, :])
            nc.scalar.dma_start(out=wkv_sb[:, sl, :], in_=wkv_d.ap()[:, sl, :])
            nc.scalar.dma_start(out=wq_sb[:, sl, :], in_=wq_d.ap()[:, sl, :])
        nc.scalar.dma_start(out=cc2_sb[:], in_=cc2_d.ap())
        nc.scalar.dma_start(out=ss2_sb[:], in_=ss2_d.ap())
        nc.scalar.dma_start(out=swp_sb[:], in_=swp_d.ap())
        nc.scalar.dma_start(
            out=maskt_sb[:], in_=maskt_d.ap().rearrange("p (b s) -> p b s", b=B)
        )
        nc.sync.dma_start(out=kct_sb[:], in_=kct_d.ap())
        nc.sync.dma_start(out=vc_sb[:], in_=vc_d.ap())
        nc.sync.dma_start(out=wo_sb[:], in_=wo_d.ap())

        # ---- on-device constants ----
        nc.gpsimd.memset(ones_sb[:], 1.0)
        make_identity(nc, ident_sb[:])

        # ---- PE warmup: back-to-back matmuls push the HAM clock gate
        # toward full rate while input DMAs stream ----
        with tc.tile_pool(name="warm_ps", bufs=1, space="PSUM") as warm_pool:
            wps = warm_pool.tile([128, 512], f32, tag="wps", name="wps")
            for _ in range(4):
                nc.tensor.matmul(
                    wps[:], warm_sb[:, 0:128], warm_sb[:],
                    start=True, stop=True, skip_group_check=True,
                )

        # ---- phase 1: QKV projection, chunk-major ----
        # One PSUM bank per accumulator: the PE's start=True reset is
        # bank-wide, so co-locating two accumulation regions in one bank
        # wipes the partner's first chunk.
        with tc.tile_pool(name="proj_ps", bufs=1, space="PSUM") as proj_pool:
            q_ps = [proj_pool.tile([128, TOK], f32, tag=f"q{h}", name=f"q{h}")
                    for h in range(NH)]
            k_ps = proj_pool.tile([128, TOK], f32, tag="k", name="k")
            v_ps = proj_pool.tile([128, TOK], f32, tag="v", name="v")

            for c in range(n_dc):
                st, sp = c == 0, c == n_dc - 1
                x_c = xt_sb[:, c, :]
                nc.tensor.matmul(k_ps[:], wkv_sb[:, c, 0:HD], x_c,
                                 start=st, stop=sp, skip_group_check=True)
                nc.tensor.matmul(v_ps[:], wkv_sb[:, c, HD: 2 * HD], x_c,
                                 start=st, stop=sp, skip_group_check=True)
                for h in range(NH):
                    nc.tensor.matmul(q_ps[h][:], wq_sb[:, c, h * HD:(h + 1) * HD],
                                     x_c, start=st, stop=sp, skip_group_check=True)

            # PSUM -> SBUF bf16 staging for rope + AV
            nc.scalar.copy(qsb[0][:, 0, :], q_ps[0][:])
            nc.scalar.copy(qsb[0][:, 1, :], q_ps[1][:])
            nc.vector.tensor_scalar_mul(ksb[:], k_ps[:], 1.0)
            vnT_sb = const.tile([128, TOK], bf, name="vnT")
            nc.vector.tensor_scalar_mul(vnT_sb[:], v_ps[:], 1.0)
            nc.scalar.copy(qsb[1][:, 0, :], q_ps[2][:])
            nc.scalar.copy(qsb[1][:, 1, :], q_ps[3][:])

        # ---- phase 1b: RoPE via PE half-swap ----
        # qT = q * [c;c] + swap(q) * [-s;s]; swap(q) comes from one matmul
        # against the host-fed half-swap permutation.
        rtmp = ctx.enter_context(tc.tile_pool(name="rope_tmp", bufs=4))
        with tc.tile_pool(name="rope_ps", bufs=1, space="PSUM") as rope_pool, \
             tc.tile_pool(name="vt_ps", bufs=2, space="PSUM") as vt_pool:
            # keep the PE hot through the Act-copy latency
            warm2 = rope_pool.tile([128, 2, TOK], f32, tag="qsw", name="warm2")
            nc.tensor.matmul(
                warm2[:].rearrange("p h t -> p (h t)"), warm_sb[:, 0:128],
                warm_sb[:], start=True, stop=True, skip_group_check=True,
            )

            def rope_pair(src_sb, dst, wide):
                """src_sb/dst: [128, 2, TOK] (pair) or [128, TOK] (k)."""
                hshape = [128, 2, TOK] if wide else [128, TOK]
                sw_ps = rope_pool.tile([128, 2, TOK], f32, tag="qsw", name="qsw")
                sw = sw_ps[:] if wide else sw_ps[:, 0, :]
                flat_in = src_sb.rearrange("p h t -> p (h t)") if wide else src_sb
                flat_sw = sw.rearrange("p h t -> p (h t)") if wide else sw
                nc.tensor.matmul(flat_sw, swp_sb[:], flat_in,
                                 start=True, stop=True, skip_group_check=True)
                cc = cc2_sb[:].unsqueeze(1).broadcast_to((128, 2, TOK)) if wide else cc2_sb[:]
                ss = ss2_sb[:].unsqueeze(1).broadcast_to((128, 2, TOK)) if wide else ss2_sb[:]
                t1 = rtmp.tile(hshape, bf, tag="rt1", name="rt1")
                t2 = rtmp.tile(hshape, bf, tag="rt2", name="rt2")
                nc.vector.tensor_mul(t1[:], src_sb, cc)
                nc.vector.tensor_mul(t2[:], sw, ss)
                nc.vector.tensor_add(dst, t1[:], t2[:])

            rope_pair(qsb[0][:, :, :], qT_sb[0][:, :, :], True)
            # V_new^T -> per-batch V_new [t=32, hd]: PE transposes fill the
            # PE while the DVE finishes pair 0's rope.
            for b in range(B):
                vt = vt_pool.tile([S, HD], bf, tag="vt", name="vt")
                nc.tensor.transpose(vt[:], vnT_sb[:, b * S:(b + 1) * S], ident_sb[:])
                nc.scalar.copy(vn_sb[:, b, :], vt[:])
            rope_pair(ksb[:], kTn_sb[:], False)
            rope_pair(qsb[1][:, :, :], qT_sb[1][:, :, :], True)

        # ---- phase 2+3: attention in two head-pair passes ----
        with tc.tile_pool(name="s_ps", bufs=3, space="PSUM") as s_pool, \
             tc.tile_pool(name="acc_ps", bufs=1, space="PSUM") as acc_pool, \
             tc.tile_pool(name="attn", bufs=6) as attn_pool, \
             tc.tile_pool(name="pair", bufs=4) as pair_pool:
            for p in range(NHP):
                qpair = qT_sb[p][:, :, :]  # [128, 2, TOK]
                o_ps = acc_pool.tile([128, 2, TOK], f32, tag="o", name="o")
                sum_ps = acc_pool.tile([1, 2 * TOK], f32, tag="sum", name="sum")
                nc.vector.memset(sacc_sb[:], 0.0)

                # new-token block first (independent of the cache loop) so
                # the normalize/exchange chain at the pass end only waits on
                # the last cache group.
                sn_ps = s_pool.tile([S, B, 2, S], f32, tag="s", name="sn")
                anp = attn_new[p][:, :, :, :]  # [S, 2, B, S]
                for b in range(B):
                    nc.tensor.matmul(
                        sn_ps[0:S, b, :, :].rearrange("p h s -> p (h s)"),
                        kTn_sb[:, b * S:(b + 1) * S],
                        qpair[:, :, b * S:(b + 1) * S], start=True, stop=True,
                        skip_group_check=True,
                    )
                nc.vector.tensor_tensor(
                    out=sn_ps[:, :, :, :],
                    in0=sn_ps[:, :, :, :],
                    in1=maskt_sb[:].unsqueeze(2).broadcast_to((S, B, 2, S)),
                    op=mybir.AluOpType.add,
                )
                nc.scalar.activation(
                    anp.rearrange("p h b s -> p b h s"), sn_ps[:, :, :, :], EXP
                )
                nc.tensor.matmul(
                    sum_ps[0:1, :], ones_sb[0:S, 0:1],
                    anp.rearrange("p h b s -> p (h b s)"),
                    start=True, stop=False, skip_group_check=True,
                )
                for b in range(B):
                    for l in range(2):
                        nc.tensor.matmul(
                            o_ps[:, l, b * S:(b + 1) * S], vn_sb[:, b, :],
                            anp[:, l, b, :],
                            start=(b == 0 and l == 0), stop=False,
                            skip_group_check=True,
                        )

                # cache loop: 2-chunk groups; PE does scores+AV; DVE
                # accumulates the exp tiles for the deferred row sums with a
                # bf16 pair/quad tree to cut SBUF traffic (the f32 carry only
                # absorbs one add per two groups). The last group's sums go
                # through PE ones-matmuls instead so the pass tail does not
                # wait on the DVE accumulation.
                work = []
                held = []  # pending even-group pair sum

                def drain_one():
                    a2, g = work.pop(0)
                    for u in range(2):
                        t = 2 * g + u
                        pn = 128 if t < n_tc - 1 else tail
                        nc.tensor.matmul(
                            o_ps[:, :, :], vc_sb[0:pn, t, :], a2[0:pn, u, :, :],
                            start=False, stop=(t == n_tc - 1),
                            skip_group_check=True,
                        )
                        if g == n_tg - 1:
                            nc.tensor.matmul(
                                sum_ps[0:1, :], ones_sb[0:pn, 0:1],
                                a2[0:pn, u, :, :].rearrange("p h t -> p (h t)"),
                                start=False, stop=False, skip_group_check=True,
                            )
                    if g < n_tg - 1:
                        tg = pair_pool.tile([128, 2, TOK], bf, tag="tg", name="tg")
                        nc.vector.tensor_add(tg[:], a2[:, 0, :, :], a2[:, 1, :, :])
                        if g == n_tg - 2:
                            nc.vector.tensor_add(sacc_sb[:], sacc_sb[:], tg[:])
                        elif held:
                            tp = held.pop()
                            tq = pair_pool.tile([128, 2, TOK], bf, tag="tq", name="tq")
                            nc.vector.tensor_add(tq[:], tp[:], tg[:])
                            nc.vector.tensor_add(sacc_sb[:], sacc_sb[:], tq[:])
                        else:
                            held.append(tg)
                    if g == n_tg - 2:
                        # convert the carry while the last group computes
                        nc.vector.tensor_scalar_mul(saccb_sb[:], sacc_sb[:], 1.0)

                for g in range(n_tg):
                    s2 = s_pool.tile([128, 2, 2, TOK], f32, tag="s", name="s")
                    for u in range(2):
                        t = 2 * g + u
                        pn = 128 if t < n_tc - 1 else tail
                        nc.tensor.matmul(
                            s2[0:pn, u, :, :], kct_sb[:, t * 128: t * 128 + pn],
                            qpair, start=True, stop=True, skip_group_check=True,
                        )
                    a2 = attn_pool.tile([128, 2, 2, TOK], bf, tag="a", name="a")
                    nc.scalar.activation(a2[:, :, :, :], s2[:, :, :, :], EXP)
                    work.append((a2, g))
                    if len(work) > 1:
                        drain_one()
                while work:
                    drain_one()

                # fold the DVE carry in via one matmul and close the sums
                nc.tensor.matmul(
                    sum_ps[0:1, :], ones_sb[0:128, 0:1],
                    saccb_sb[:].rearrange("p h t -> p (h t)"),
                    start=False, stop=True, skip_group_check=True,
                )

                # 1/rowsum -> broadcast -> normalize on PSUM->SBUF copy
                nc.vector.reciprocal_approx_fast(recip_sb[p][:], sum_ps[0:1, :])
                nc.vector.tensor_scalar_mul(recipb_sb[p][:], recip_sb[p][:], 1.0)
                nc.gpsimd.partition_broadcast(recip_bc[p][:], recipb_sb[p][:])
                nc.vector.tensor_mul(
                    attnout[p][:], o_ps[:, :, :].rearrange("p h t -> p (h t)"),
                    recip_bc[p][:],
                )

                # all-gather this pass's attention outputs
                nc.scalar.dma_start(ag_in[p][:], attnout[p][:])
                nc.gpsimd.collective_compute(
                    "AllGather", mybir.AluOpType.bypass,
                    replica_groups=[list(range(cores))],
                    ins=[ag_in[p].opt()], outs=[ag_out[p].opt()],
                )
                # per-source readback pieces on alternating queues: the Wo
                # block for source j unblocks as soon as its piece lands
                ag_r = ag_out[p].rearrange("(r p) n -> p r n", p=128)
                for j in range(cores):
                    eng = nc.sync if j % 2 == 0 else nc.scalar
                    eng.dma_start(allx[p][:, j, :], ag_r[:, j, :])

        # ---- phase 4: out = attnout_all @ Wo[:, slice], per pass ----
        # Wo row blocks are ordered (j, p, l) = head 4j + 2p + l on the
        # host. Pass-0's half runs as soon as its exchange lands (while
        # pass-1 attention streams); pass-1's half waits only on its own
        # exchange.
        with tc.tile_pool(name="wo_ps", bufs=1, space="PSUM") as wo_pool:
            out_ps = [wo_pool.tile([128, outc], f32, tag=f"out{k}", name=f"out{k}")
                      for k in range(2)]
            out_r = out_d.ap().rearrange("(k p) n -> p k n", p=128)
            for p in range(NHP):
                for k in range(2):
                    for j in range(cores):
                        for l in range(2):
                            g = j * NH + 2 * p + l
                            nc.tensor.matmul(
                                out_ps[k][:],
                                allx[p][:, j, l * TOK + k * 128: l * TOK + k * 128 + 128],
                                wo_sb[:, g, :],
                                start=(p == 0 and j == 0 and l == 0),
                                stop=(p == NHP - 1 and j == cores - 1 and l == 1),
                                skip_group_check=True,
                            )
                    if p == NHP - 1:
                        nc.scalar.copy(out_sb[:, k, :], out_ps[k][:])
                        nc.sync.dma_start(out_r[:, k, :], out_sb[:, k, :])

    nc.compile()
    return nc


def _pack(a):
    return np.ascontiguousarray(a)


def prep_in_maps(x, freqs_cos, freqs_sin, mask, cache_k, cache_v, Wq, Wk, Wv, Wo,
                 cores=CORES, gather="cc"):
    """Host-side sharding + partition-major packing."""
    n_dc = D // 128
    n_tc = (PREV + 127) // 128
    outc = D // cores

    x = np.asarray(x, np.float32).reshape(TOK, D)
    xt = x.T.astype(BF16)  # [D, TOK]
    xt_p = _pack(xt.reshape(n_dc, 128, TOK).transpose(1, 0, 2))  # [128, n_dc, TOK]
    cost = np.tile(np.asarray(freqs_cos, np.float32)[0].T, (1, B))  # [64, TOK]
    sint = np.tile(np.asarray(freqs_sin, np.float32)[0].T, (1, B))
    cc2 = _pack(np.concatenate([cost, cost], axis=0).astype(BF16))  # [128, TOK]
    ss2 = _pack(np.concatenate([-sint, sint], axis=0).astype(BF16))
    swp = np.zeros((128, 128), np.float32)
    swp[(np.arange(128) + 64) % 128, np.arange(128)] = 1.0
    swp = _pack(swp.astype(BF16))
    maskt = _pack(np.asarray(mask, np.float32).transpose(2, 0, 1).reshape(S, TOK))
    Wq = np.asarray(Wq, np.float32)
    Wk = np.asarray(Wk, np.float32)
    Wv = np.asarray(Wv, np.float32)
    Wo = np.asarray(Wo, np.float32)
    cache_k = np.asarray(cache_k, np.float32)
    cache_v = np.asarray(cache_v, np.float32)

    in_maps = []
    for c in range(cores):
        wq_c = (Wq[:, c * QCOLS:(c + 1) * QCOLS] * SCALE).reshape(D, NH, HD)[
            :, :, _IDX
        ].reshape(D, QCOLS).astype(BF16)
        wq_p = _pack(wq_c.reshape(n_dc, 128, QCOLS).transpose(1, 0, 2))
        wk_c = Wk[:, c * HD:(c + 1) * HD][:, _IDX]
        wv_c = Wv[:, c * HD:(c + 1) * HD]
        wkv_c = np.concatenate([wk_c, wv_c], axis=1).astype(BF16)
        wkv_p = _pack(wkv_c.reshape(n_dc, 128, 2 * HD).transpose(1, 0, 2))
        kct_c = _pack(cache_k[0, :PREV, c, :][:, _IDX].T.astype(BF16))  # [HD, PREV]
        vc_full = np.zeros((n_tc * 128, HD), np.float32)
        vc_full[:PREV] = cache_v[0, :PREV, c, :]
        vc_p = _pack(vc_full.astype(BF16).reshape(n_tc, 128, HD).transpose(1, 0, 2))
        # Wo rows ordered: block (j, p, l) = head 4j + 2p + l
        wo_c = Wo[:, c * outc:(c + 1) * outc].astype(BF16)  # [H*HD, outc]
        wo_blocks = wo_c.reshape(H, HD, outc)
        order = [4 * j + 2 * p + l
                 for j in range(cores) for p in range(NHP) for l in range(2)]
        wo_x = wo_blocks[order]  # [32, HD, outc]
        wo_p = _pack(wo_x.transpose(1, 0, 2))  # [128, H, outc]
        in_maps.append(
            {
                "xt": xt_p,
                "wq": wq_p,
                "wkv": wkv_p,
                "kct": kct_c,
                "vc": vc_p,
                "wo": wo_p,
                "cc2": cc2,
                "ss2": ss2,
                "swp": swp,
                "maskt": maskt,
            }
        )
    return in_maps


def kernel(x, freqs_cos, freqs_sin, mask, cache_k, cache_v, Wq, Wk, Wv, Wo, positions):
    global LAST_EXEC_NS, LAST_RES
    assert int(positions) == PREV, f"kernel compiled for positions={PREV}"

    key = ("v3", os.environ.get("KERNEL_GATHER", "cc"))
    if key not in _BUILD_CACHE:
        _BUILD_CACHE[key] = build(CORES, key[1])
    nc = _BUILD_CACHE[key]

    in_maps = prep_in_maps(
        x, freqs_cos, freqs_sin, mask, cache_k, cache_v, Wq, Wk, Wv, Wo,
        CORES, key[1]
    )

    trace = os.environ.get("KERNEL_TRACE", "0") == "1"
    if trace:
        _install_ntff_hook()
    res = run_bass_kernel_spmd(
        nc, in_maps, core_ids=list(range(CORES)), trace=trace
    )
    if trace:
        LAST_EXEC_NS = res.exec_time_ns
        LAST_RES = res

    outc = D // CORES
    out = np.empty((TOK, D), np.float32)
    for c in range(CORES):
        out[:, c * outc:(c + 1) * outc] = res.results[c]["out"]
    return out.reshape(B, S, D)


# revision 25
# speedup vs baseline: 1.0170x; 1.0170x over previous
"""Trainium2 Bass kernel for GQA sliding-window attention (8-core SPMD).

Problem: B=8, S=32, D=4096, H=32 Q-heads, KVH=8 KV-heads, HD=128,
sliding window 4096 with 4064 cached positions.

Sharding: tensor-parallel over heads. Core c owns Q heads 4c..4c+3 and KV
head c (one GQA group): Wq/Wk/Wv column-sharded, cache sharded by KV head,
x replicated. Attention runs in two head-pair passes; after each pass the
cores all-gather that pass's (bf16) attention outputs through the runtime
collective stream, then each core applies a column slice of Wo; the host
concatenates column slices.

Layout/numerics notes:
  - All inputs are host-packed partition-major so every DMA lands with
    per-partition-contiguous multi-KB descriptor runs.
  - x is fed transposed (xT) so Q/K projections produce Q^T/K^T directly
    in [head_dim, token] layout. V is projected with xT chunks as the
    stationary operand instead, yielding V_new directly in [token, hd]
    layout (no PE transposes).
  - Wq/Wk columns (and cached K's hd axis) are permuted so RoPE's
    interleaved (even,odd) pairs become contiguous halves. The permutation
    cancels in q.k. SCALE is folded into Wq.
  - RoPE runs as qT = q * [c;c] + swap(q) * [-s;s] where swap exchanges
    the real/imag partition halves via one PE matmul against a host-fed
    128x128 permutation matrix. Three wide DVE ops per head pair replace
    the old 6-op/head serial chain.
  - Softmax skips max-subtraction; normalization is deferred: exp tiles
    are accumulated chunk-wise on the Vector engine into a [128, 512] f32
    carry, one ones-matmul per pass reduces it across partitions, and
    1/sum is applied when copying attention outputs out of PSUM.
  - Scores for two cache chunks accumulate into one 2-bank PSUM tile so a
    single ACTIVATE exps 2 chunks (halves Act instruction overhead).
"""

import os
import sys
from contextlib import ExitStack

import numpy as np
import ml_dtypes

import concourse.bass as bass
import concourse.tile as tile
import concourse.mybir as mybir
from concourse import bacc
from concourse.bass_utils import run_bass_kernel_spmd
from concourse.masks import make_identity

BF16 = ml_dtypes.bfloat16

CORES = 8
B, S, D = 8, 32, 4096
H, KVH, HD = 32, 8, 128
SW = 4096
PREV = SW - S  # 4064
TOK = B * S  # 256
NH = H // KVH  # 4 Q heads per core
NHP = NH // 2  # head pairs per core
QCOLS = NH * HD  # 512 Q-projection columns per core
SCALE = float(HD) ** -0.5

# hd permutation: interleaved (r0,i0,r1,i1,...) -> (r..., i...)
_IDX = np.concatenate([np.arange(0, HD, 2), np.arange(1, HD, 2)])

# exec time of the last traced run (ns), set when KERNEL_TRACE=1
LAST_EXEC_NS = None
LAST_RES = None

_BUILD_CACHE = {}


def _install_ntff_hook():
    """Register the axon NTFF profiling hook (the agent image's antenv stub
    lacks axon_hooks). Only needed when tracing."""
    import types

    if "antenv.axon_hooks" in sys.modules:
        return
    try:
        from trn_agent_boot.trn_boot import _ntff_profile_via_ctypes

        hook = _ntff_profile_via_ctypes("/opt/axon/libaxon_pjrt.so")
    except Exception:
        hook = None
    mod = types.ModuleType("antenv.axon_hooks")
    mod._hook = hook
    mod.get_axon_ntff_profile_hook = lambda: mod._hook
    mod.set_axon_ntff_profile_hook = lambda h: setattr(mod, "_hook", h)
    sys.modules["antenv.axon_hooks"] = mod
    import antenv

    antenv.axon_hooks = mod


def build(cores=CORES, gather=None):
    gather = gather or os.environ.get("KERNEL_GATHER", "cc")
    assert gather == "cc"
    n_dc = D // 128  # 32 contraction chunks for QKV projections
    n_tc = (PREV + 127) // 128  # cache t-chunks (last short)
    tail = PREV - (n_tc - 1) * 128  # 96
    outc = D // cores  # Wo output columns per core
    n_xp = 4  # xt/wq DMA pieces
    xp = n_dc // n_xp
    n_tg = n_tc // 2  # 2-chunk score/exp groups

    dt = mybir.dt
    bf, f32 = dt.bfloat16, dt.float32
    EXP = mybir.ActivationFunctionType.Exp

    nc = bacc.Bacc("TRN2", target_bir_lowering=False, debug=False, num_devices=cores)

    xt_d = nc.dram_tensor("xt", [128, n_dc, TOK], bf, kind="ExternalInput")
    wq_d = nc.dram_tensor("wq", [128, n_dc, QCOLS], bf, kind="ExternalInput")
    wkv_d = nc.dram_tensor("wkv", [128, n_dc, 2 * HD], bf, kind="ExternalInput")
    kct_d = nc.dram_tensor("kct", [HD, PREV], bf, kind="ExternalInput")
    vc_d = nc.dram_tensor("vc", [128, n_tc, HD], bf, kind="ExternalInput")
    wo_d = nc.dram_tensor("wo", [128, H, outc], bf, kind="ExternalInput")
    cc2_d = nc.dram_tensor("cc2", [128, TOK], bf, kind="ExternalInput")
    ss2_d = nc.dram_tensor("ss2", [128, TOK], bf, kind="ExternalInput")
    swp_d = nc.dram_tensor("swp", [128, 128], bf, kind="ExternalInput")
    maskt_d = nc.dram_tensor("maskt", [S, TOK], f32, kind="ExternalInput")
    out_d = nc.dram_tensor("out", [TOK, outc], f32, kind="ExternalOutput")

    with tile.TileContext(nc) as tc, ExitStack() as ctx:
        const = ctx.enter_context(tc.tile_pool(name="const", bufs=1))

        xt_sb = const.tile([128, n_dc, TOK], bf)
        wq_sb = const.tile([128, n_dc, QCOLS], bf)
        wkv_sb = const.tile([128, n_dc, 2 * HD], bf)
        kct_sb = const.tile([128, PREV], bf)
        vc_sb = const.tile([128, n_tc, HD], bf)
        wo_sb = const.tile([128, H, outc], bf)
        cc2_sb = const.tile([128, TOK], bf)
        ss2_sb = const.tile([128, TOK], bf)
        swp_sb = const.tile([128, 128], bf)
        maskt_sb = const.tile([S, B, S], f32)
        ones_sb = const.tile([128, 1], bf)
        ident_sb = const.tile([128, 128], bf)
        qsb = [const.tile([128, 2, TOK], bf, tag=f"qsb{p}", name=f"qsb{p}") for p in range(NHP)]
        ksb = const.tile([128, TOK], bf)
        qT_sb = [const.tile([128, 2, TOK], bf, tag=f"qT{p}", name=f"qT{p}") for p in range(NHP)]
        kTn_sb = const.tile([128, TOK], bf)
        vn_sb = const.tile([S, B, HD], bf)
        attn_new = [const.tile([S, 2, B, S], bf, tag=f"an{p}", name=f"an{p}") for p in range(NHP)]
        sacc_sb = const.tile([128, 2, TOK], f32, name="sacc")
        saccb_sb = const.tile([128, 2, TOK], bf, name="saccb")
        recip_sb = [const.tile([1, 2 * TOK], f32, tag=f"rc{p}", name=f"rc{p}") for p in range(NHP)]
        recipb_sb = [const.tile([1, 2 * TOK], bf, tag=f"rcb{p}", name=f"rcb{p}") for p in range(NHP)]
        recip_bc = [const.tile([128, 2 * TOK], bf, tag=f"rb{p}", name=f"rb{p}") for p in range(NHP)]
        attnout = [const.tile([128, 2 * TOK], bf, tag=f"ao{p}", name=f"ao{p}") for p in range(NHP)]
        allx = [
            const.tile([128, cores, 2 * TOK], bf, tag=f"all{p}", name=f"all{p}")
            for p in range(NHP)
        ]
        out_sb = const.tile([128, 2, outc], f32, name="out_sb")
        warm_sb = const.tile([128, 512], bf, name="warm_sb")

        # ---- cross-core launch sync: fire the tiny AllGather as early as
        # possible so the collective-stream rendezvous (which absorbs core
        # launch skew) overlaps the input DMA + projection phase. ----
        dram = ctx.enter_context(tc.tile_pool(name="dram", bufs=1, space="DRAM"))
        ag_in = [dram.tile([128, 2 * TOK], bf, tag=f"agi{p}", name=f"agi{p}") for p in range(NHP)]
        ag_out = [
            dram.tile([128 * cores, 2 * TOK], bf, tag=f"ago{p}", name=f"ago{p}",
                      addr_space="Shared")
            for p in range(NHP)
        ]

        nc.vector.memset(warm_sb[:], 0.0)

        # ---- input DMAs ----
        # All projection-critical bytes ride the sync queue (it starts
        # delivering ~10us earlier than the scalar queue), interleaved in
        # consumption order so the chunk loop paces with no cross-queue
        # contention. The slow-starting scalar queue takes everything the
        # attention phase needs (nothing before ~40us).
        for i in range(n_xp):
            sl = slice(i * xp, (i + 1) * xp)
            nc.sync.dma_start(out=xt_sb[:, sl, :], in_=xt_d.ap()[:, sl, :])
            nc.sync.dma_start(out=wkv_sb[:, sl, :], in_=wkv_d.ap()[:, sl, :])
            nc.sync.dma_start(out=wq_sb[:, sl, :], in_=wq_d.ap()[:, sl, :])
        nc.scalar.dma_start(out=cc2_sb[:], in_=cc2_d.ap())
        nc.scalar.dma_start(out=ss2_sb[:], in_=ss2_d.ap())
        nc.scalar.dma_start(out=swp_sb[:], in_=swp_d.ap())
        nc.scalar.dma_start(
            out=maskt_sb[:], in_=maskt_d.ap().rearrange("p (b s) -> p b s", b=B)
        )
        nc.scalar.dma_start(out=kct_sb[:], in_=kct_d.ap())
        nc.scalar.dma_start(out=vc_sb[:], in_=vc_d.ap())
        nc.scalar.dma_start(out=wo_sb[:], in_=wo_d.ap())

        # ---- on-device constants ----
        nc.gpsimd.memset(ones_sb[:], 1.0)
        make_identity(nc, ident_sb[:])

        # ---- PE warmup: back-to-back matmuls push the HAM clock gate
        # toward full rate while input DMAs stream ----
        with tc.tile_pool(name="warm_ps", bufs=1, space="PSUM") as warm_pool:
            wps = warm_pool.tile([128, 512], f32, tag="wps", name="wps")
            for _ in range(4):
                nc.tensor.matmul(
                    wps[:], warm_sb[:, 0:128], warm_sb[:],
                    start=True, stop=True, skip_group_check=True,
                )

        # ---- phase 1: QKV projection, chunk-major ----
        # One PSUM bank per accumulator: the PE's start=True reset is
        # bank-wide, so co-locating two accumulation regions in one bank
        # wipes the partner's first chunk.
        with tc.tile_pool(name="proj_ps", bufs=1, space="PSUM") as proj_pool:
            q_ps = [proj_pool.tile([128, TOK], f32, tag=f"q{h}", name=f"q{h}")
                    for h in range(NH)]
            k_ps = proj_pool.tile([128, TOK], f32, tag="k", name="k")
            v_ps = proj_pool.tile([128, TOK], f32, tag="v", name="v")

            for c in range(n_dc):
                st, sp = c == 0, c == n_dc - 1
                x_c = xt_sb[:, c, :]
                nc.tensor.matmul(k_ps[:], wkv_sb[:, c, 0:HD], x_c,
                                 start=st, stop=sp, skip_group_check=True)
                nc.tensor.matmul(v_ps[:], wkv_sb[:, c, HD: 2 * HD], x_c,
                                 start=st, stop=sp, skip_group_check=True)
                for h in range(NH):
                    nc.tensor.matmul(q_ps[h][:], wq_sb[:, c, h * HD:(h + 1) * HD],
                                     x_c, start=st, stop=sp, skip_group_check=True)

            # PSUM -> SBUF bf16 staging for rope + AV
            nc.scalar.copy(qsb[0][:, 0, :], q_ps[0][:])
            nc.scalar.copy(qsb[0][:, 1, :], q_ps[1][:])
            nc.vector.tensor_scalar_mul(ksb[:], k_ps[:], 1.0)
            vnT_sb = const.tile([128, TOK], bf, name="vnT")
            nc.vector.tensor_scalar_mul(vnT_sb[:], v_ps[:], 1.0)
            nc.scalar.copy(qsb[1][:, 0, :], q_ps[2][:])
            nc.scalar.copy(qsb[1][:, 1, :], q_ps[3][:])

        # ---- phase 1b: RoPE via PE half-swap ----
        # qT = q * [c;c] + swap(q) * [-s;s]; swap(q) comes from one matmul
        # against the host-fed half-swap permutation.
        rtmp = ctx.enter_context(tc.tile_pool(name="rope_tmp", bufs=4))
        with tc.tile_pool(name="rope_ps", bufs=1, space="PSUM") as rope_pool, \
             tc.tile_pool(name="vt_ps", bufs=2, space="PSUM") as vt_pool:
            # keep the PE hot through the Act-copy latency
            warm2 = rope_pool.tile([128, 2, TOK], f32, tag="qsw", name="warm2")
            nc.tensor.matmul(
                warm2[:].rearrange("p h t -> p (h t)"), warm_sb[:, 0:128],
                warm_sb[:], start=True, stop=True, skip_group_check=True,
            )

            def rope_pair(src_sb, dst, wide):
                """src_sb/dst: [128, 2, TOK] (pair) or [128, TOK] (k)."""
                hshape = [128, 2, TOK] if wide else [128, TOK]
                sw_ps = rope_pool.tile([128, 2, TOK], f32, tag="qsw", name="qsw")
                sw = sw_ps[:] if wide else sw_ps[:, 0, :]
                flat_in = src_sb.rearrange("p h t -> p (h t)") if wide else src_sb
                flat_sw = sw.rearrange("p h t -> p (h t)") if wide else sw
                nc.tensor.matmul(flat_sw, swp_sb[:], flat_in,
                                 start=True, stop=True, skip_group_check=True)
                cc = cc2_sb[:].unsqueeze(1).broadcast_to((128, 2, TOK)) if wide else cc2_sb[:]
                ss = ss2_sb[:].unsqueeze(1).broadcast_to((128, 2, TOK)) if wide else ss2_sb[:]
                t1 = rtmp.tile(hshape, bf, tag="rt1", name="rt1")
                t2 = rtmp.tile(hshape, bf, tag="rt2", name="rt2")
                nc.vector.tensor_mul(t1[:], src_sb, cc)
                nc.vector.tensor_mul(t2[:], sw, ss)
                nc.vector.tensor_add(dst, t1[:], t2[:])

            rope_pair(qsb[0][:, :, :], qT_sb[0][:, :, :], True)
            # V_new^T -> per-batch V_new [t=32, hd]: PE transposes fill the
            # PE while the DVE finishes pair 0's rope.
            for b in range(B):
                vt = vt_pool.tile([S, HD], bf, tag="vt", name="vt")
                nc.tensor.transpose(vt[:], vnT_sb[:, b * S:(b + 1) * S], ident_sb[:])
                nc.scalar.copy(vn_sb[:, b, :], vt[:])
            rope_pair(ksb[:], kTn_sb[:], False)
            rope_pair(qsb[1][:, :, :], qT_sb[1][:, :, :], True)

        # ---- phase 2+3: attention in two head-pair passes ----
        with tc.tile_pool(name="s_ps", bufs=3, space="PSUM") as s_pool, \
             tc.tile_pool(name="acc_ps", bufs=1, space="PSUM") as acc_pool, \
             tc.tile_pool(name="attn", bufs=6) as attn_pool, \
             tc.tile_pool(name="pair", bufs=4) as pair_pool:
            for p in range(NHP):
                qpair = qT_sb[p][:, :, :]  # [128, 2, TOK]
                o_ps = acc_pool.tile([128, 2, TOK], f32, tag="o", name="o")
                sum_ps = acc_pool.tile([1, 2 * TOK], f32, tag="sum", name="sum")
                nc.vector.memset(sacc_sb[:], 0.0)

                # new-token block first (independent of the cache loop) so
                # the normalize/exchange chain at the pass end only waits on
                # the last cache group.
                sn_ps = s_pool.tile([S, B, 2, S], f32, tag="s", name="sn")
                anp = attn_new[p][:, :, :, :]  # [S, 2, B, S]
                for b in range(B):
                    nc.tensor.matmul(
                        sn_ps[0:S, b, :, :].rearrange("p h s -> p (h s)"),
                        kTn_sb[:, b * S:(b + 1) * S],
                        qpair[:, :, b * S:(b + 1) * S], start=True, stop=True,
                        skip_group_check=True,
                    )
                nc.vector.tensor_tensor(
                    out=sn_ps[:, :, :, :],
                    in0=sn_ps[:, :, :, :],
                    in1=maskt_sb[:].unsqueeze(2).broadcast_to((S, B, 2, S)),
                    op=mybir.AluOpType.add,
                )
                nc.scalar.activation(
                    anp.rearrange("p h b s -> p b h s"), sn_ps[:, :, :, :], EXP
                )
                nc.tensor.matmul(
                    sum_ps[0:1, :], ones_sb[0:S, 0:1],
                    anp.rearrange("p h b s -> p (h b s)"),
                    start=True, stop=False, skip_group_check=True,
                )
                for b in range(B):
                    for l in range(2):
                        nc.tensor.matmul(
                            o_ps[:, l, b * S:(b + 1) * S], vn_sb[:, b, :],
                            anp[:, l, b, :],
                            start=(b == 0 and l == 0), stop=False,
                            skip_group_check=True,
                        )

                # cache loop: 2-chunk groups; PE does scores+AV; DVE
                # accumulates the exp tiles for the deferred row sums with a
                # bf16 pair/quad tree to cut SBUF traffic (the f32 carry only
                # absorbs one add per two groups). The last group's sums go
                # through PE ones-matmuls instead so the pass tail does not
                # wait on the DVE accumulation.
                work = []
                held = []  # pending even-group pair sum

                def drain_one():
                    a2, g = work.pop(0)
                    for u in range(2):
                        t = 2 * g + u
                        pn = 128 if t < n_tc - 1 else tail
                        nc.tensor.matmul(
                            o_ps[:, :, :], vc_sb[0:pn, t, :], a2[0:pn, u, :, :],
                            start=False, stop=(t == n_tc - 1),
                            skip_group_check=True,
                        )
                        if g == n_tg - 1:
                            nc.tensor.matmul(
                                sum_ps[0:1, :], ones_sb[0:pn, 0:1],
                                a2[0:pn, u, :, :].rearrange("p h t -> p (h t)"),
                                start=False, stop=False, skip_group_check=True,
                            )
                    if g < n_tg - 1:
                        tg = pair_pool.tile([128, 2, TOK], bf, tag="tg", name="tg")
                        nc.vector.tensor_add(tg[:], a2[:, 0, :, :], a2[:, 1, :, :])
                        if g == n_tg - 2:
                            nc.vector.tensor_add(sacc_sb[:], sacc_sb[:], tg[:])
                        elif held:
                            tp = held.pop()
                            tq = pair_pool.tile([128, 2, TOK], bf, tag="tq", name="tq")
                            nc.vector.tensor_add(tq[:], tp[:], tg[:])
                            nc.vector.tensor_add(sacc_sb[:], sacc_sb[:], tq[:])
                        else:
                            held.append(tg)
                    if g == n_tg - 2:
                        # convert the carry while the last group computes
                        nc.vector.tensor_scalar_mul(saccb_sb[:], sacc_sb[:], 1.0)

                for g in range(n_tg):
                    s2 = s_pool.tile([128, 2, 2, TOK], f32, tag="s", name="s")
                    for u in range(2):
                        t = 2 * g + u
                        pn = 128 if t < n_tc - 1 else tail
                        nc.tensor.matmul(
                            s2[0:pn, u, :, :], kct_sb[:, t * 128: t * 128 + pn],
                            qpair, start=True, stop=True, skip_group_check=True,
                        )
                    a2 = attn_pool.tile([128, 2, 2, TOK], bf, tag="a", name="a")
                    nc.scalar.activation(a2[:, :, :, :], s2[:, :, :, :], EXP)
                    work.append((a2, g))
                    if len(work) > 1:
                        drain_one()
                while work:
                    drain_one()

                # fold the DVE carry in via one matmul and close the sums
                nc.tensor.matmul(
                    sum_ps[0:1, :], ones_sb[0:128, 0:1],
                    saccb_sb[:].rearrange("p h t -> p (h t)"),
                    start=False, stop=True, skip_group_check=True,
                )

                # 1/rowsum -> broadcast -> normalize on PSUM->SBUF copy
                nc.vector.reciprocal_approx_fast(recip_sb[p][:], sum_ps[0:1, :])
                nc.vector.tensor_scalar_mul(recipb_sb[p][:], recip_sb[p][:], 1.0)
                nc.gpsimd.partition_broadcast(recip_bc[p][:], recipb_sb[p][:])
                nc.vector.tensor_mul(
                    attnout[p][:], o_ps[:, :, :].rearrange("p h t -> p (h t)"),
                    recip_bc[p][:],
                )

                # all-gather this pass's attention outputs
                nc.scalar.dma_start(ag_in[p][:], attnout[p][:])
                nc.gpsimd.collective_compute(
                    "AllGather", mybir.AluOpType.bypass,
                    replica_groups=[list(range(cores))],
                    ins=[ag_in[p].opt()], outs=[ag_out[p].opt()],
                )
                # per-source readback pieces on alternating queues: the Wo
                # block for source j unblocks as soon as its piece lands
                ag_r = ag_out[p].rearrange("(r p) n -> p r n", p=128)
                for j in range(cores):
                    eng = nc.sync if j % 2 == 0 else nc.scalar
                    eng.dma_start(allx[p][:, j, :], ag_r[:, j, :])

        # ---- phase 4: out = attnout_all @ Wo[:, slice], per pass ----
        # Wo row blocks are ordered (j, p, l) = head 4j + 2p + l on the
        # host. Pass-0's half runs as soon as its exchange lands (while
        # pass-1 attention streams); pass-1's half waits only on its own
        # exchange.
        with tc.tile_pool(name="wo_ps", bufs=1, space="PSUM") as wo_pool:
            out_ps = [wo_pool.tile([128, outc], f32, tag=f"out{k}", name=f"out{k}")
                      for k in range(2)]
            out_r = out_d.ap().rearrange("(k p) n -> p k n", p=128)
            for p in range(NHP):
                for k in range(2):
                    for j in range(cores):
                        for l in range(2):
                            g = j * NH + 2 * p + l
                            nc.tensor.matmul(
                                out_ps[k][:],
                                allx[p][:, j, l * TOK + k * 128: l * TOK + k * 128 + 128],
                                wo_sb[:, g, :],
                                start=(p == 0 and j == 0 and l == 0),
                                stop=(p == NHP - 1 and j == cores - 1 and l == 1),
                                skip_group_check=True,
                            )
                    if p == NHP - 1:
                        nc.scalar.copy(out_sb[:, k, :], out_ps[k][:])
                        nc.sync.dma_start(out_r[:, k, :], out_sb[:, k, :])

    nc.compile()
    return nc


def _pack(a):
    return np.ascontiguousarray(a)


def prep_in_maps(x, freqs_cos, freqs_sin, mask, cache_k, cache_v, Wq, Wk, Wv, Wo,
                 cores=CORES, gather="cc"):
    """Host-side sharding + partition-major packing."""
    n_dc = D // 128
    n_tc = (PREV + 127) // 128
    outc = D // cores

    x = np.asarray(x, np.float32).reshape(TOK, D)
    xt = x.T.astype(BF16)  # [D, TOK]
    xt_p = _pack(xt.reshape(n_dc, 128, TOK).transpose(1, 0, 2))  # [128, n_dc, TOK]
    cost = np.tile(np.asarray(freqs_cos, np.float32)[0].T, (1, B))  # [64, TOK]
    sint = np.tile(np.asarray(freqs_sin, np.float32)[0].T, (1, B))
    cc2 = _pack(np.concatenate([cost, cost], axis=0).astype(BF16))  # [128, TOK]
    ss2 = _pack(np.concatenate([-sint, sint], axis=0).astype(BF16))
    swp = np.zeros((128, 128), np.float32)
    swp[(np.arange(128) + 64) % 128, np.arange(128)] = 1.0
    swp = _pack(swp.astype(BF16))
    maskt = _pack(np.asarray(mask, np.float32).transpose(2, 0, 1).reshape(S, TOK))
    Wq = np.asarray(Wq, np.float32)
    Wk = np.asarray(Wk, np.float32)
    Wv = np.asarray(Wv, np.float32)
    Wo = np.asarray(Wo, np.float32)
    cache_k = np.asarray(cache_k, np.float32)
    cache_v = np.asarray(cache_v, np.float32)

    in_maps = []
    for c in range(cores):
        wq_c = (Wq[:, c * QCOLS:(c + 1) * QCOLS] * SCALE).reshape(D, NH, HD)[
            :, :, _IDX
        ].reshape(D, QCOLS).astype(BF16)
        wq_p = _pack(wq_c.reshape(n_dc, 128, QCOLS).transpose(1, 0, 2))
        wk_c = Wk[:, c * HD:(c + 1) * HD][:, _IDX]
        wv_c = Wv[:, c * HD:(c + 1) * HD]
        wkv_c = np.concatenate([wk_c, wv_c], axis=1).astype(BF16)
        wkv_p = _pack(wkv_c.reshape(n_dc, 128, 2 * HD).transpose(1, 0, 2))
        kct_c = _pack(cache_k[0, :PREV, c, :][:, _IDX].T.astype(BF16))  # [HD, PREV]
        vc_full = np.zeros((n_tc * 128, HD), np.float32)
        vc_full[:PREV] = cache_v[0, :PREV, c, :]
        vc_p = _pack(vc_full.astype(BF16).reshape(n_tc, 128, HD).transpose(1, 0, 2))
        # Wo rows ordered: block (j, p, l) = head 4j + 2p + l
        wo_c = Wo[:, c * outc:(c + 1) * outc].astype(BF16)  # [H*HD, outc]
        wo_blocks = wo_c.reshape(H, HD, outc)
        order = [4 * j + 2 * p + l
                 for j in range(cores) for p in range(NHP) for l in range(2)]
        wo_x = wo_blocks[order]  # [32, HD, outc]
        wo_p = _pack(wo_x.transpose(1, 0, 2))  # [128, H, outc]
        in_maps.append(
            {
                "xt": xt_p,
                "wq": wq_p,
                "wkv": wkv_p,
                "kct": kct_c,
                "vc": vc_p,
                "wo": wo_p,
                "cc2": cc2,
                "ss2": ss2,
                "swp": swp,
                "maskt": maskt,
            }
        )
    return in_maps


def kernel(x, freqs_cos, freqs_sin, mask, cache_k, cache_v, Wq, Wk, Wv, Wo, positions):
    global LAST_EXEC_NS, LAST_RES
    assert int(positions) == PREV, f"kernel compiled for positions={PREV}"

    key = ("v3", os.environ.get("KERNEL_GATHER", "cc"))
    if key not in _BUILD_CACHE:
        _BUILD_CACHE[key] = build(CORES, key[1])
    nc = _BUILD_CACHE[key]

    in_maps = prep_in_maps(
        x, freqs_cos, freqs_sin, mask, cache_k, cache_v, Wq, Wk, Wv, Wo,
        CORES, key[1]
    )

    trace = os.environ.get("KERNEL_TRACE", "0") == "1"
    if trace:
        _install_ntff_hook()
    res = run_bass_kernel_spmd(
        nc, in_maps, core_ids=list(range(CORES)), trace=trace
    )
    if trace:
        LAST_EXEC_NS = res.exec_time_ns
        LAST_RES = res

    outc = D // CORES
    out = np.empty((TOK, D), np.float32)
    for c in range(CORES):
        out[:, c * outc:(c + 1) * outc] = res.results[c]["out"]
    return out.reshape(B, S, D)


# revision 27
# speedup vs baseline: 1.1427x; 1.1235x over previous
"""Trainium2 Bass kernel for GQA sliding-window attention (8-core SPMD).

Problem: B=8, S=32, D=4096, H=32 Q-heads, KVH=8 KV-heads, HD=128,
sliding window 4096 with 4064 cached positions.

Sharding: tensor-parallel over heads. Core c owns Q heads 4c..4c+3 and KV
head c (one GQA group): Wq/Wk/Wv column-sharded, cache sharded by KV head,
x replicated. Attention runs in two head-pair passes; after each pass the
cores all-gather that pass's (bf16) attention outputs through the runtime
collective stream, then each core applies a column slice of Wo; the host
concatenates column slices.

Layout/numerics notes:
  - All inputs are host-packed partition-major so every DMA lands with
    per-partition-contiguous multi-KB descriptor runs.
  - x is fed transposed (xT) so Q/K projections produce Q^T/K^T directly
    in [head_dim, token] layout. V is projected with xT chunks as the
    stationary operand instead, yielding V_new directly in [token, hd]
    layout (no PE transposes).
  - Wq/Wk columns (and cached K's hd axis) are permuted so RoPE's
    interleaved (even,odd) pairs become contiguous halves. The permutation
    cancels in q.k. SCALE is folded into Wq.
  - RoPE runs as qT = q * [c;c] + swap(q) * [-s;s] where swap exchanges
    the real/imag partition halves via one PE matmul against a host-fed
    128x128 permutation matrix. Three wide DVE ops per head pair replace
    the old 6-op/head serial chain.
  - Softmax skips max-subtraction; normalization is deferred: exp tiles
    are accumulated chunk-wise on the Vector engine into a [128, 512] f32
    carry, one ones-matmul per pass reduces it across partitions, and
    1/sum is applied when copying attention outputs out of PSUM.
  - Scores for two cache chunks accumulate into one 2-bank PSUM tile so a
    single ACTIVATE exps 2 chunks (halves Act instruction overhead).
"""

import os
import sys
from contextlib import ExitStack

import numpy as np
import ml_dtypes

import concourse.bass as bass
import concourse.tile as tile
import concourse.mybir as mybir
from concourse import bacc
from concourse.bass_utils import run_bass_kernel_spmd
from concourse.masks import make_identity

BF16 = ml_dtypes.bfloat16

CORES = 8
B, S, D = 8, 32, 4096
H, KVH, HD = 32, 8, 128
SW = 4096
PREV = SW - S  # 4064
TOK = B * S  # 256
NH = H // KVH  # 4 Q heads per core
NHP = NH // 2  # head pairs per core
QCOLS = NH * HD  # 512 Q-projection columns per core
SCALE = float(HD) ** -0.5

# hd permutation: interleaved (r0,i0,r1,i1,...) -> (r..., i...)
_IDX = np.concatenate([np.arange(0, HD, 2), np.arange(1, HD, 2)])

# exec time of the last traced run (ns), set when KERNEL_TRACE=1
LAST_EXEC_NS = None
LAST_RES = None

_BUILD_CACHE = {}


def _install_ntff_hook():
    """Register the axon NTFF profiling hook (the agent image's antenv stub
    lacks axon_hooks). Only needed when tracing."""
    import types

    if "antenv.axon_hooks" in sys.modules:
        return
    try:
        from trn_agent_boot.trn_boot import _ntff_profile_via_ctypes

        hook = _ntff_profile_via_ctypes("/opt/axon/libaxon_pjrt.so")
    except Exception:
        hook = None
    mod = types.ModuleType("antenv.axon_hooks")
    mod._hook = hook
    mod.get_axon_ntff_profile_hook = lambda: mod._hook
    mod.set_axon_ntff_profile_hook = lambda h: setattr(mod, "_hook", h)
    sys.modules["antenv.axon_hooks"] = mod
    import antenv

    antenv.axon_hooks = mod


def build(cores=CORES, gather=None):
    gather = gather or os.environ.get("KERNEL_GATHER", "cc")
    assert gather == "cc"
    n_dc = D // 128  # 32 contraction chunks for QKV projections
    n_tc = (PREV + 127) // 128  # cache t-chunks (last short)
    tail = PREV - (n_tc - 1) * 128  # 96
    outc = D // cores  # Wo output columns per core
    n_xp = 4  # xt/wq DMA pieces
    xp = n_dc // n_xp
    n_tg = n_tc // 2  # 2-chunk score/exp groups

    dt = mybir.dt
    bf, f32 = dt.bfloat16, dt.float32
    EXP = mybir.ActivationFunctionType.Exp

    nc = bacc.Bacc("TRN2", target_bir_lowering=False, debug=False, num_devices=cores)

    xt_d = nc.dram_tensor("xt", [128, n_dc, TOK], bf, kind="ExternalInput")
    wq_d = nc.dram_tensor("wq", [128, n_dc, QCOLS], bf, kind="ExternalInput")
    wkv_d = nc.dram_tensor("wkv", [128, n_dc, 2 * HD], bf, kind="ExternalInput")
    kct_d = nc.dram_tensor("kct", [HD, PREV], bf, kind="ExternalInput")
    vc_d = nc.dram_tensor("vc", [128, n_tc, HD], bf, kind="ExternalInput")
    wo_d = nc.dram_tensor("wo", [128, H, outc], bf, kind="ExternalInput")
    cc2_d = nc.dram_tensor("cc2", [128, TOK], bf, kind="ExternalInput")
    ss2_d = nc.dram_tensor("ss2", [128, TOK], bf, kind="ExternalInput")
    swp_d = nc.dram_tensor("swp", [128, 128], bf, kind="ExternalInput")
    maskt_d = nc.dram_tensor("maskt", [S, TOK], f32, kind="ExternalInput")
    out_d = nc.dram_tensor("out", [TOK, outc], f32, kind="ExternalOutput")

    with tile.TileContext(nc) as tc, ExitStack() as ctx:
        const = ctx.enter_context(tc.tile_pool(name="const", bufs=1))

        xt_sb = const.tile([128, n_dc, TOK], bf)
        wq_sb = const.tile([128, n_dc, QCOLS], bf)
        wkv_sb = const.tile([128, n_dc, 2 * HD], bf)
        kct_sb = const.tile([128, PREV], bf)
        vc_sb = const.tile([128, n_tc, HD], bf)
        wo_sb = const.tile([128, H, outc], bf)
        cc2_sb = const.tile([128, TOK], bf)
        ss2_sb = const.tile([128, TOK], bf)
        swp_sb = const.tile([128, 128], bf)
        maskt_sb = const.tile([S, B, S], f32)
        ones_sb = const.tile([128, 1], bf)
        ident_sb = const.tile([128, 128], bf)
        qsb = [const.tile([128, 2, TOK], bf, tag=f"qsb{p}", name=f"qsb{p}") for p in range(NHP)]
        ksb = const.tile([128, TOK], bf)
        qT_sb = [const.tile([128, 2, TOK], bf, tag=f"qT{p}", name=f"qT{p}") for p in range(NHP)]
        kTn_sb = const.tile([128, TOK], bf)
        vn_sb = const.tile([S, B, HD], bf)
        attn_new = [const.tile([S, 2, B, S], bf, tag=f"an{p}", name=f"an{p}") for p in range(NHP)]
        sacc_sb = const.tile([128, 2, TOK], f32, name="sacc")
        saccb_sb = const.tile([128, 2, TOK], bf, name="saccb")
        recip_sb = [const.tile([1, 2 * TOK], f32, tag=f"rc{p}", name=f"rc{p}") for p in range(NHP)]
        recipb_sb = [const.tile([1, 2 * TOK], bf, tag=f"rcb{p}", name=f"rcb{p}") for p in range(NHP)]
        recip_bc = [const.tile([128, 2 * TOK], bf, tag=f"rb{p}", name=f"rb{p}") for p in range(NHP)]
        attnout = [const.tile([128, 2 * TOK], bf, tag=f"ao{p}", name=f"ao{p}") for p in range(NHP)]
        allx = [
            const.tile([128, cores, 2 * TOK], bf, tag=f"all{p}", name=f"all{p}")
            for p in range(NHP)
        ]
        out_sb = const.tile([128, 2, outc], f32, name="out_sb")
        warm_sb = const.tile([128, 512], bf, name="warm_sb")

        # ---- cross-core launch sync: fire the tiny AllGather as early as
        # possible so the collective-stream rendezvous (which absorbs core
        # launch skew) overlaps the input DMA + projection phase. ----
        dram = ctx.enter_context(tc.tile_pool(name="dram", bufs=1, space="DRAM"))
        ag_in = [dram.tile([128, 2 * TOK], bf, tag=f"agi{p}", name=f"agi{p}") for p in range(NHP)]
        ag_out = [
            dram.tile([128 * cores, 2 * TOK], bf, tag=f"ago{p}", name=f"ago{p}",
                      addr_space="Shared")
            for p in range(NHP)
        ]

        nc.vector.memset(warm_sb[:], 0.0)

        # ---- input DMAs ----
        # EVERY large input rides the sync queue (the early-starting one),
        # strictly in consumption order — two busy queues just split the
        # same HBM bandwidth and starve whichever tensor is needed first.
        # The slow-starting scalar queue gets only the small tables.
        for i in range(n_xp):
            sl = slice(i * xp, (i + 1) * xp)
            nc.sync.dma_start(out=xt_sb[:, sl, :], in_=xt_d.ap()[:, sl, :])
            nc.sync.dma_start(out=wkv_sb[:, sl, :], in_=wkv_d.ap()[:, sl, :])
            nc.sync.dma_start(out=wq_sb[:, sl, :], in_=wq_d.ap()[:, sl, :])
        nc.scalar.dma_start(out=cc2_sb[:], in_=cc2_d.ap())
        nc.scalar.dma_start(out=ss2_sb[:], in_=ss2_d.ap())
        nc.scalar.dma_start(out=swp_sb[:], in_=swp_d.ap())
        nc.scalar.dma_start(
            out=maskt_sb[:], in_=maskt_d.ap().rearrange("p (b s) -> p b s", b=B)
        )
        nc.sync.dma_start(out=kct_sb[:], in_=kct_d.ap())
        nc.sync.dma_start(out=vc_sb[:], in_=vc_d.ap())
        nc.sync.dma_start(out=wo_sb[:], in_=wo_d.ap())

        # ---- on-device constants ----
        nc.gpsimd.memset(ones_sb[:], 1.0)
        make_identity(nc, ident_sb[:])

        # ---- PE warmup: back-to-back matmuls push the HAM clock gate
        # toward full rate while input DMAs stream ----
        with tc.tile_pool(name="warm_ps", bufs=1, space="PSUM") as warm_pool:
            wps = warm_pool.tile([128, 512], f32, tag="wps", name="wps")
            for _ in range(4):
                nc.tensor.matmul(
                    wps[:], warm_sb[:, 0:128], warm_sb[:],
                    start=True, stop=True, skip_group_check=True,
                )

        # ---- phase 1: QKV projection, chunk-major ----
        # One PSUM bank per accumulator: the PE's start=True reset is
        # bank-wide, so co-locating two accumulation regions in one bank
        # wipes the partner's first chunk.
        with tc.tile_pool(name="proj_ps", bufs=1, space="PSUM") as proj_pool:
            q_ps = [proj_pool.tile([128, TOK], f32, tag=f"q{h}", name=f"q{h}")
                    for h in range(NH)]
            k_ps = proj_pool.tile([128, TOK], f32, tag="k", name="k")
            v_ps = proj_pool.tile([128, TOK], f32, tag="v", name="v")

            for c in range(n_dc):
                st, sp = c == 0, c == n_dc - 1
                x_c = xt_sb[:, c, :]
                nc.tensor.matmul(k_ps[:], wkv_sb[:, c, 0:HD], x_c,
                                 start=st, stop=sp, skip_group_check=True)
                nc.tensor.matmul(v_ps[:], wkv_sb[:, c, HD: 2 * HD], x_c,
                                 start=st, stop=sp, skip_group_check=True)
                for h in range(NH):
                    nc.tensor.matmul(q_ps[h][:], wq_sb[:, c, h * HD:(h + 1) * HD],
                                     x_c, start=st, stop=sp, skip_group_check=True)

            # PSUM -> SBUF bf16 staging for rope + AV
            nc.scalar.copy(qsb[0][:, 0, :], q_ps[0][:])
            nc.scalar.copy(qsb[0][:, 1, :], q_ps[1][:])
            nc.vector.tensor_scalar_mul(ksb[:], k_ps[:], 1.0)
            vnT_sb = const.tile([128, TOK], bf, name="vnT")
            nc.vector.tensor_scalar_mul(vnT_sb[:], v_ps[:], 1.0)
            nc.scalar.copy(qsb[1][:, 0, :], q_ps[2][:])
            nc.scalar.copy(qsb[1][:, 1, :], q_ps[3][:])

        # ---- phase 1b: RoPE via PE half-swap ----
        # qT = q * [c;c] + swap(q) * [-s;s]; swap(q) comes from one matmul
        # against the host-fed half-swap permutation.
        rtmp = ctx.enter_context(tc.tile_pool(name="rope_tmp", bufs=4))
        with tc.tile_pool(name="rope_ps", bufs=1, space="PSUM") as rope_pool, \
             tc.tile_pool(name="vt_ps", bufs=2, space="PSUM") as vt_pool:
            # keep the PE hot through the Act-copy latency
            warm2 = rope_pool.tile([128, 2, TOK], f32, tag="qsw", name="warm2")
            nc.tensor.matmul(
                warm2[:].rearrange("p h t -> p (h t)"), warm_sb[:, 0:128],
                warm_sb[:], start=True, stop=True, skip_group_check=True,
            )

            def rope_pair(src_sb, dst, wide):
                """src_sb/dst: [128, 2, TOK] (pair) or [128, TOK] (k)."""
                hshape = [128, 2, TOK] if wide else [128, TOK]
                sw_ps = rope_pool.tile([128, 2, TOK], f32, tag="qsw", name="qsw")
                sw = sw_ps[:] if wide else sw_ps[:, 0, :]
                flat_in = src_sb.rearrange("p h t -> p (h t)") if wide else src_sb
                flat_sw = sw.rearrange("p h t -> p (h t)") if wide else sw
                nc.tensor.matmul(flat_sw, swp_sb[:], flat_in,
                                 start=True, stop=True, skip_group_check=True)
                cc = cc2_sb[:].unsqueeze(1).broadcast_to((128, 2, TOK)) if wide else cc2_sb[:]
                ss = ss2_sb[:].unsqueeze(1).broadcast_to((128, 2, TOK)) if wide else ss2_sb[:]
                t1 = rtmp.tile(hshape, bf, tag="rt1", name="rt1")
                t2 = rtmp.tile(hshape, bf, tag="rt2", name="rt2")
                nc.vector.tensor_mul(t1[:], src_sb, cc)
                nc.vector.tensor_mul(t2[:], sw, ss)
                nc.vector.tensor_add(dst, t1[:], t2[:])

            rope_pair(qsb[0][:, :, :], qT_sb[0][:, :, :], True)
            # V_new^T -> per-batch V_new [t=32, hd]: PE transposes fill the
            # PE while the DVE finishes pair 0's rope.
            for b in range(B):
                vt = vt_pool.tile([S, HD], bf, tag="vt", name="vt")
                nc.tensor.transpose(vt[:], vnT_sb[:, b * S:(b + 1) * S], ident_sb[:])
                nc.scalar.copy(vn_sb[:, b, :], vt[:])
            rope_pair(ksb[:], kTn_sb[:], False)
            rope_pair(qsb[1][:, :, :], qT_sb[1][:, :, :], True)

        # ---- phase 2+3: attention in two head-pair passes ----
        with tc.tile_pool(name="s_ps", bufs=3, space="PSUM") as s_pool, \
             tc.tile_pool(name="acc_ps", bufs=1, space="PSUM") as acc_pool, \
             tc.tile_pool(name="attn", bufs=6) as attn_pool, \
             tc.tile_pool(name="pair", bufs=4) as pair_pool:
            for p in range(NHP):
                qpair = qT_sb[p][:, :, :]  # [128, 2, TOK]
                o_ps = acc_pool.tile([128, 2, TOK], f32, tag="o", name="o")
                sum_ps = acc_pool.tile([1, 2 * TOK], f32, tag="sum", name="sum")
                nc.vector.memset(sacc_sb[:], 0.0)

                # new-token block first (independent of the cache loop) so
                # the normalize/exchange chain at the pass end only waits on
                # the last cache group.
                sn_ps = s_pool.tile([S, B, 2, S], f32, tag="s", name="sn")
                anp = attn_new[p][:, :, :, :]  # [S, 2, B, S]
                for b in range(B):
                    nc.tensor.matmul(
                        sn_ps[0:S, b, :, :].rearrange("p h s -> p (h s)"),
                        kTn_sb[:, b * S:(b + 1) * S],
                        qpair[:, :, b * S:(b + 1) * S], start=True, stop=True,
                        skip_group_check=True,
                    )
                nc.vector.tensor_tensor(
                    out=sn_ps[:, :, :, :],
                    in0=sn_ps[:, :, :, :],
                    in1=maskt_sb[:].unsqueeze(2).broadcast_to((S, B, 2, S)),
                    op=mybir.AluOpType.add,
                )
                nc.scalar.activation(
                    anp.rearrange("p h b s -> p b h s"), sn_ps[:, :, :, :], EXP
                )
                nc.tensor.matmul(
                    sum_ps[0:1, :], ones_sb[0:S, 0:1],
                    anp.rearrange("p h b s -> p (h b s)"),
                    start=True, stop=False, skip_group_check=True,
                )
                for b in range(B):
                    for l in range(2):
                        nc.tensor.matmul(
                            o_ps[:, l, b * S:(b + 1) * S], vn_sb[:, b, :],
                            anp[:, l, b, :],
                            start=(b == 0 and l == 0), stop=False,
                            skip_group_check=True,
                        )

                # cache loop: 2-chunk groups; PE does scores+AV; DVE
                # accumulates the exp tiles for the deferred row sums with a
                # bf16 pair/quad tree to cut SBUF traffic (the f32 carry only
                # absorbs one add per two groups). The last group's sums go
                # through PE ones-matmuls instead so the pass tail does not
                # wait on the DVE accumulation.
                work = []
                held = []  # pending even-group pair sum

                def drain_one():
                    a2, g = work.pop(0)
                    for u in range(2):
                        t = 2 * g + u
                        pn = 128 if t < n_tc - 1 else tail
                        nc.tensor.matmul(
                            o_ps[:, :, :], vc_sb[0:pn, t, :], a2[0:pn, u, :, :],
                            start=False, stop=(t == n_tc - 1),
                            skip_group_check=True,
                        )
                        if g == n_tg - 1:
                            nc.tensor.matmul(
                                sum_ps[0:1, :], ones_sb[0:pn, 0:1],
                                a2[0:pn, u, :, :].rearrange("p h t -> p (h t)"),
                                start=False, stop=False, skip_group_check=True,
                            )
                    if g < n_tg - 1:
                        tg = pair_pool.tile([128, 2, TOK], bf, tag="tg", name="tg")
                        nc.vector.tensor_add(tg[:], a2[:, 0, :, :], a2[:, 1, :, :])
                        if g == n_tg - 2:
                            nc.vector.tensor_add(sacc_sb[:], sacc_sb[:], tg[:])
                        elif held:
                            tp = held.pop()
                            tq = pair_pool.tile([128, 2, TOK], bf, tag="tq", name="tq")
                            nc.vector.tensor_add(tq[:], tp[:], tg[:])
                            nc.vector.tensor_add(sacc_sb[:], sacc_sb[:], tq[:])
                        else:
                            held.append(tg)
                    if g == n_tg - 2:
                        # convert the carry while the last group computes
                        nc.vector.tensor_scalar_mul(saccb_sb[:], sacc_sb[:], 1.0)

                for g in range(n_tg):
                    s2 = s_pool.tile([128, 2, 2, TOK], f32, tag="s", name="s")
                    for u in range(2):
                        t = 2 * g + u
                        pn = 128 if t < n_tc - 1 else tail
                        nc.tensor.matmul(
                            s2[0:pn, u, :, :], kct_sb[:, t * 128: t * 128 + pn],
                            qpair, start=True, stop=True, skip_group_check=True,
                        )
                    a2 = attn_pool.tile([128, 2, 2, TOK], bf, tag="a", name="a")
                    nc.scalar.activation(a2[:, :, :, :], s2[:, :, :, :], EXP)
                    work.append((a2, g))
                    if len(work) > 1:
                        drain_one()
                while work:
                    drain_one()

                # fold the DVE carry in via one matmul and close the sums
                nc.tensor.matmul(
                    sum_ps[0:1, :], ones_sb[0:128, 0:1],
                    saccb_sb[:].rearrange("p h t -> p (h t)"),
                    start=False, stop=True, skip_group_check=True,
                )

                # 1/rowsum -> broadcast -> normalize on PSUM->SBUF copy
                nc.vector.reciprocal_approx_fast(recip_sb[p][:], sum_ps[0:1, :])
                nc.vector.tensor_scalar_mul(recipb_sb[p][:], recip_sb[p][:], 1.0)
                nc.gpsimd.partition_broadcast(recip_bc[p][:], recipb_sb[p][:])
                nc.vector.tensor_mul(
                    attnout[p][:], o_ps[:, :, :].rearrange("p h t -> p (h t)"),
                    recip_bc[p][:],
                )

                # all-gather this pass's attention outputs. The ag_in fill
                # rides the sync queue and the readback pieces ride the
                # scalar queue: a shared queue would head-of-line-block
                # pass 1's fill behind pass 0's readback (which waits on
                # the collective), serializing the exchanges.
                nc.sync.dma_start(ag_in[p][:], attnout[p][:])
                nc.gpsimd.collective_compute(
                    "AllGather", mybir.AluOpType.bypass,
                    replica_groups=[list(range(cores))],
                    ins=[ag_in[p].opt()], outs=[ag_out[p].opt()],
                )
                # per-source readback pieces: the Wo block for source j
                # unblocks as soon as its piece lands
                ag_r = ag_out[p].rearrange("(r p) n -> p r n", p=128)
                for j in range(cores):
                    nc.scalar.dma_start(allx[p][:, j, :], ag_r[:, j, :])

        # ---- phase 4: out = attnout_all @ Wo[:, slice], per pass ----
        # Wo row blocks are ordered (j, p, l) = head 4j + 2p + l on the
        # host. Pass-0's half runs as soon as its exchange lands (while
        # pass-1 attention streams); pass-1's half waits only on its own
        # exchange.
        with tc.tile_pool(name="wo_ps", bufs=1, space="PSUM") as wo_pool:
            out_ps = [wo_pool.tile([128, outc], f32, tag=f"out{k}", name=f"out{k}")
                      for k in range(2)]
            out_r = out_d.ap().rearrange("(k p) n -> p k n", p=128)
            for p in range(NHP):
                for k in range(2):
                    for j in range(cores):
                        for l in range(2):
                            g = j * NH + 2 * p + l
                            nc.tensor.matmul(
                                out_ps[k][:],
                                allx[p][:, j, l * TOK + k * 128: l * TOK + k * 128 + 128],
                                wo_sb[:, g, :],
                                start=(p == 0 and j == 0 and l == 0),
                                stop=(p == NHP - 1 and j == cores - 1 and l == 1),
                                skip_group_check=True,
                            )
                    if p == NHP - 1:
                        nc.scalar.copy(out_sb[:, k, :], out_ps[k][:])
                        nc.sync.dma_start(out_r[:, k, :], out_sb[:, k, :])

    nc.compile()
    return nc


def _pack(a):
    return np.ascontiguousarray(a)


def prep_in_maps(x, freqs_cos, freqs_sin, mask, cache_k, cache_v, Wq, Wk, Wv, Wo,
                 cores=CORES, gather="cc"):
    """Host-side sharding + partition-major packing."""
    n_dc = D // 128
    n_tc = (PREV + 127) // 128
    outc = D // cores

    x = np.asarray(x, np.float32).reshape(TOK, D)
    xt = x.T.astype(BF16)  # [D, TOK]
    xt_p = _pack(xt.reshape(n_dc, 128, TOK).transpose(1, 0, 2))  # [128, n_dc, TOK]
    cost = np.tile(np.asarray(freqs_cos, np.float32)[0].T, (1, B))  # [64, TOK]
    sint = np.tile(np.asarray(freqs_sin, np.float32)[0].T, (1, B))
    cc2 = _pack(np.concatenate([cost, cost], axis=0).astype(BF16))  # [128, TOK]
    ss2 = _pack(np.concatenate([-sint, sint], axis=0).astype(BF16))
    swp = np.zeros((128, 128), np.float32)
    swp[(np.arange(128) + 64) % 128, np.arange(128)] = 1.0
    swp = _pack(swp.astype(BF16))
    maskt = _pack(np.asarray(mask, np.float32).transpose(2, 0, 1).reshape(S, TOK))
    Wq = np.asarray(Wq, np.float32)
    Wk = np.asarray(Wk, np.float32)
    Wv = np.asarray(Wv, np.float32)
    Wo = np.asarray(Wo, np.float32)
    cache_k = np.asarray(cache_k, np.float32)
    cache_v = np.asarray(cache_v, np.float32)

    in_maps = []
    for c in range(cores):
        wq_c = (Wq[:, c * QCOLS:(c + 1) * QCOLS] * SCALE).reshape(D, NH, HD)[
            :, :, _IDX
        ].reshape(D, QCOLS).astype(BF16)
        wq_p = _pack(wq_c.reshape(n_dc, 128, QCOLS).transpose(1, 0, 2))
        wk_c = Wk[:, c * HD:(c + 1) * HD][:, _IDX]
        wv_c = Wv[:, c * HD:(c + 1) * HD]
        wkv_c = np.concatenate([wk_c, wv_c], axis=1).astype(BF16)
        wkv_p = _pack(wkv_c.reshape(n_dc, 128, 2 * HD).transpose(1, 0, 2))
        kct_c = _pack(cache_k[0, :PREV, c, :][:, _IDX].T.astype(BF16))  # [HD, PREV]
        vc_full = np.zeros((n_tc * 128, HD), np.float32)
        vc_full[:PREV] = cache_v[0, :PREV, c, :]
        vc_p = _pack(vc_full.astype(BF16).reshape(n_tc, 128, HD).transpose(1, 0, 2))
        # Wo rows ordered: block (j, p, l) = head 4j + 2p + l
        wo_c = Wo[:, c * outc:(c + 1) * outc].astype(BF16)  # [H*HD, outc]
        wo_blocks = wo_c.reshape(H, HD, outc)
        order = [4 * j + 2 * p + l
                 for j in range(cores) for p in range(NHP) for l in range(2)]
        wo_x = wo_blocks[order]  # [32, HD, outc]
        wo_p = _pack(wo_x.transpose(1, 0, 2))  # [128, H, outc]
        in_maps.append(
            {
                "xt": xt_p,
                "wq": wq_p,
                "wkv": wkv_p,
                "kct": kct_c,
                "vc": vc_p,
                "wo": wo_p,
                "cc2": cc2,
                "ss2": ss2,
                "swp": swp,
                "maskt": maskt,
            }
        )
    return in_maps


def kernel(x, freqs_cos, freqs_sin, mask, cache_k, cache_v, Wq, Wk, Wv, Wo, positions):
    global LAST_EXEC_NS, LAST_RES
    assert int(positions) == PREV, f"kernel compiled for positions={PREV}"

    key = ("v3", os.environ.get("KERNEL_GATHER", "cc"))
    if key not in _BUILD_CACHE:
        _BUILD_CACHE[key] = build(CORES, key[1])
    nc = _BUILD_CACHE[key]

    in_maps = prep_in_maps(
        x, freqs_cos, freqs_sin, mask, cache_k, cache_v, Wq, Wk, Wv, Wo,
        CORES, key[1]
    )

    trace = os.environ.get("KERNEL_TRACE", "0") == "1"
    if trace:
        _install_ntff_hook()
    res = run_bass_kernel_spmd(
        nc, in_maps, core_ids=list(range(CORES)), trace=trace
    )
    if trace:
        LAST_EXEC_NS = res.exec_time_ns
        LAST_RES = res

    outc = D // CORES
    out = np.empty((TOK, D), np.float32)
    for c in range(CORES):
        out[:, c * outc:(c + 1) * outc] = res.results[c]["out"]
    return out.reshape(B, S, D)


# revision 31
# speedup vs baseline: 1.2351x; 1.0809x over previous
"""Trainium2 Bass kernel for GQA sliding-window attention (8-core SPMD).

Problem: B=8, S=32, D=4096, H=32 Q-heads, KVH=8 KV-heads, HD=128,
sliding window 4096 with 4064 cached positions.

Sharding: tensor-parallel over heads. Core c owns Q heads 4c..4c+3 and KV
head c (one GQA group): Wq/Wk/Wv column-sharded, cache sharded by KV head,
x replicated. Attention runs in two head-pair passes; after each pass the
cores all-gather that pass's (bf16) attention outputs through the runtime
collective stream, then each core applies a column slice of Wo; the host
concatenates column slices.

Layout/numerics notes:
  - All inputs are host-packed partition-major so every DMA lands with
    per-partition-contiguous multi-KB descriptor runs.
  - x is fed transposed (xT) so Q/K projections produce Q^T/K^T directly
    in [head_dim, token] layout. V is projected with xT chunks as the
    stationary operand instead, yielding V_new directly in [token, hd]
    layout (no PE transposes).
  - Wq/Wk columns (and cached K's hd axis) are permuted so RoPE's
    interleaved (even,odd) pairs become contiguous halves. The permutation
    cancels in q.k. SCALE is folded into Wq.
  - RoPE runs as qT = q * [c;c] + swap(q) * [-s;s] where swap exchanges
    the real/imag partition halves via one PE matmul against a host-fed
    128x128 permutation matrix. Three wide DVE ops per head pair replace
    the old 6-op/head serial chain.
  - Softmax skips max-subtraction; normalization is deferred: exp tiles
    are accumulated chunk-wise on the Vector engine into a [128, 512] f32
    carry, one ones-matmul per pass reduces it across partitions, and
    1/sum is applied when copying attention outputs out of PSUM.
  - Scores for two cache chunks accumulate into one 2-bank PSUM tile so a
    single ACTIVATE exps 2 chunks (halves Act instruction overhead).
"""

import os
import sys
from contextlib import ExitStack

import numpy as np
import ml_dtypes

import concourse.bass as bass
import concourse.tile as tile
import concourse.mybir as mybir
from concourse import bacc
from concourse.bass_utils import run_bass_kernel_spmd
from concourse.masks import make_identity

BF16 = ml_dtypes.bfloat16

CORES = 8
B, S, D = 8, 32, 4096
H, KVH, HD = 32, 8, 128
SW = 4096
PREV = SW - S  # 4064
TOK = B * S  # 256
NH = H // KVH  # 4 Q heads per core
NHP = NH // 2  # head pairs per core
QCOLS = NH * HD  # 512 Q-projection columns per core
SCALE = float(HD) ** -0.5

# hd permutation: interleaved (r0,i0,r1,i1,...) -> (r..., i...)
_IDX = np.concatenate([np.arange(0, HD, 2), np.arange(1, HD, 2)])

# exec time of the last traced run (ns), set when KERNEL_TRACE=1
LAST_EXEC_NS = None
LAST_RES = None

_BUILD_CACHE = {}


def _install_ntff_hook():
    """Register the axon NTFF profiling hook (the agent image's antenv stub
    lacks axon_hooks). Only needed when tracing."""
    import types

    if "antenv.axon_hooks" in sys.modules:
        return
    try:
        from trn_agent_boot.trn_boot import _ntff_profile_via_ctypes

        hook = _ntff_profile_via_ctypes("/opt/axon/libaxon_pjrt.so")
    except Exception:
        hook = None
    mod = types.ModuleType("antenv.axon_hooks")
    mod._hook = hook
    mod.get_axon_ntff_profile_hook = lambda: mod._hook
    mod.set_axon_ntff_profile_hook = lambda h: setattr(mod, "_hook", h)
    sys.modules["antenv.axon_hooks"] = mod
    import antenv

    antenv.axon_hooks = mod


def build(cores=CORES, gather=None):
    gather = gather or os.environ.get("KERNEL_GATHER", "cc")
    assert gather == "cc"
    n_dc = D // 128  # 32 contraction chunks for QKV projections
    n_tc = (PREV + 127) // 128  # cache t-chunks (last short)
    tail = PREV - (n_tc - 1) * 128  # 96
    outc = D // cores  # Wo output columns per core
    n_xp = 4  # xt/wq DMA pieces
    xp = n_dc // n_xp
    n_tg = n_tc // 2  # 2-chunk score/exp groups

    dt = mybir.dt
    bf, f32 = dt.bfloat16, dt.float32
    EXP = mybir.ActivationFunctionType.Exp

    nc = bacc.Bacc("TRN2", target_bir_lowering=False, debug=False, num_devices=cores)

    xt_d = nc.dram_tensor("xt", [128, n_dc, TOK], bf, kind="ExternalInput")
    wq_d = nc.dram_tensor("wq", [128, n_dc, QCOLS], bf, kind="ExternalInput")
    wkv_d = nc.dram_tensor("wkv", [128, n_dc, 2 * HD], bf, kind="ExternalInput")
    kct_d = nc.dram_tensor("kct", [HD, PREV], bf, kind="ExternalInput")
    vc_d = nc.dram_tensor("vc", [128, n_tc, HD], bf, kind="ExternalInput")
    wo_d = nc.dram_tensor("wo", [128, H, outc], bf, kind="ExternalInput")
    cc2_d = nc.dram_tensor("cc2", [128, TOK], bf, kind="ExternalInput")
    ss2_d = nc.dram_tensor("ss2", [128, TOK], bf, kind="ExternalInput")
    swp_d = nc.dram_tensor("swp", [128, 128], bf, kind="ExternalInput")
    maskt_d = nc.dram_tensor("maskt", [S, TOK], f32, kind="ExternalInput")
    out_d = nc.dram_tensor("out", [TOK, outc], f32, kind="ExternalOutput")

    with tile.TileContext(nc) as tc, ExitStack() as ctx:
        const = ctx.enter_context(tc.tile_pool(name="const", bufs=1))

        xt_sb = const.tile([128, n_dc, TOK], bf)
        wq_sb = const.tile([128, n_dc, QCOLS], bf)
        wkv_sb = const.tile([128, n_dc, 2 * HD], bf)
        kct_sb = const.tile([128, PREV], bf)
        vc_sb = const.tile([128, n_tc, HD], bf)
        wo_sb = const.tile([128, H, outc], bf)
        cc2_sb = const.tile([128, TOK], bf)
        ss2_sb = const.tile([128, TOK], bf)
        swp_sb = const.tile([128, 128], bf)
        maskt_sb = const.tile([S, B, S], f32)
        ones_sb = const.tile([128, 1], bf)
        ident_sb = const.tile([128, 128], bf)
        qsb = [const.tile([128, 2, TOK], bf, tag=f"qsb{p}", name=f"qsb{p}") for p in range(NHP)]
        ksb = const.tile([128, TOK], bf)
        qT_sb = [const.tile([128, 2, TOK], bf, tag=f"qT{p}", name=f"qT{p}") for p in range(NHP)]
        kTn_sb = const.tile([128, TOK], bf)
        vn_sb = const.tile([S, B, HD], bf)
        attn_new = [const.tile([S, 2, B, S], bf, tag=f"an{p}", name=f"an{p}") for p in range(NHP)]
        sacc_sb = const.tile([128, 2, TOK], f32, name="sacc")
        saccb_sb = const.tile([128, 2, TOK], bf, name="saccb")
        recip_sb = [const.tile([1, 2 * TOK], f32, tag=f"rc{p}", name=f"rc{p}") for p in range(NHP)]
        recipb_sb = [const.tile([1, 2 * TOK], bf, tag=f"rcb{p}", name=f"rcb{p}") for p in range(NHP)]
        recip_bc = [const.tile([128, 2 * TOK], bf, tag=f"rb{p}", name=f"rb{p}") for p in range(NHP)]
        attnout = [const.tile([128, 2 * TOK], bf, tag=f"ao{p}", name=f"ao{p}") for p in range(NHP)]
        allx = [
            const.tile([128, cores, 2 * TOK], bf, tag=f"all{p}", name=f"all{p}")
            for p in range(NHP)
        ]
        out_sb = const.tile([128, 2, outc], f32, name="out_sb")
        warm_sb = const.tile([128, 512], bf, name="warm_sb")

        # ---- cross-core launch sync: fire the tiny AllGather as early as
        # possible so the collective-stream rendezvous (which absorbs core
        # launch skew) overlaps the input DMA + projection phase. ----
        dram = ctx.enter_context(tc.tile_pool(name="dram", bufs=1, space="DRAM"))
        agw_in = dram.tile([1, 64], bf, name="agw_in")
        agw_out = dram.tile([cores, 64], bf, name="agw_out", addr_space="Shared")
        ag_in = [dram.tile([128, 2 * TOK], bf, tag=f"agi{p}", name=f"agi{p}") for p in range(NHP)]
        ag_out = [
            dram.tile([128 * cores, 2 * TOK], bf, tag=f"ago{p}", name=f"ago{p}",
                      addr_space="Shared")
            for p in range(NHP)
        ]

        nc.vector.memset(warm_sb[:], 0.0)
        # Warm-up collective: fires immediately (DRAM->DRAM fill, no compute
        # producer) and absorbs both the core-launch rendezvous and the CC
        # stream's expensive first-op setup while projections stream — the
        # real exchanges then start ~1us after their trigger.
        nc.scalar.dma_start(out=agw_in[:], in_=cc2_d.ap()[0:1, 0:64])
        nc.gpsimd.collective_compute(
            "AllGather", mybir.AluOpType.bypass,
            replica_groups=[list(range(cores))],
            ins=[agw_in.opt()], outs=[agw_out.opt()],
        )

        # ---- input DMAs ----
        # EVERY large input rides the sync queue (the early-starting one),
        # strictly in consumption order — two busy queues just split the
        # same HBM bandwidth and starve whichever tensor is needed first.
        # The slow-starting scalar queue gets only the small tables.
        bounds = [0, 4, 12, 22, 32]  # small first piece: projections start sooner
        for i in range(len(bounds) - 1):
            sl = slice(bounds[i], bounds[i + 1])
            nc.sync.dma_start(out=xt_sb[:, sl, :], in_=xt_d.ap()[:, sl, :])
            nc.sync.dma_start(out=wkv_sb[:, sl, :], in_=wkv_d.ap()[:, sl, :])
            nc.sync.dma_start(out=wq_sb[:, sl, :], in_=wq_d.ap()[:, sl, :])
        nc.scalar.dma_start(out=cc2_sb[:], in_=cc2_d.ap())
        nc.scalar.dma_start(out=ss2_sb[:], in_=ss2_d.ap())
        nc.scalar.dma_start(out=swp_sb[:], in_=swp_d.ap())
        nc.scalar.dma_start(
            out=maskt_sb[:], in_=maskt_d.ap().rearrange("p (b s) -> p b s", b=B)
        )
        nc.sync.dma_start(out=kct_sb[:], in_=kct_d.ap())
        nc.sync.dma_start(out=vc_sb[:], in_=vc_d.ap())
        nc.sync.dma_start(out=wo_sb[:], in_=wo_d.ap())

        # ---- on-device constants ----
        nc.gpsimd.memset(ones_sb[:], 1.0)
        make_identity(nc, ident_sb[:])

        # ---- PE warmup: back-to-back matmuls push the HAM clock gate
        # toward full rate while input DMAs stream ----
        with tc.tile_pool(name="warm_ps", bufs=1, space="PSUM") as warm_pool:
            wps = warm_pool.tile([128, 512], f32, tag="wps", name="wps")
            for _ in range(2):
                nc.tensor.matmul(
                    wps[:], warm_sb[:, 0:128], warm_sb[:],
                    start=True, stop=True, skip_group_check=True,
                )

        # ---- phase 1: QKV projection, chunk-major ----
        # One PSUM bank per accumulator: the PE's start=True reset is
        # bank-wide, so co-locating two accumulation regions in one bank
        # wipes the partner's first chunk.
        with tc.tile_pool(name="proj_ps", bufs=1, space="PSUM") as proj_pool:
            q_ps = [proj_pool.tile([128, TOK], f32, tag=f"q{h}", name=f"q{h}")
                    for h in range(NH)]
            k_ps = proj_pool.tile([128, TOK], f32, tag="k", name="k")
            v_ps = proj_pool.tile([128, TOK], f32, tag="v", name="v")

            for c in range(n_dc):
                st, sp = c == 0, c == n_dc - 1
                x_c = xt_sb[:, c, :]
                nc.tensor.matmul(k_ps[:], wkv_sb[:, c, 0:HD], x_c,
                                 start=st, stop=sp, skip_group_check=True)
                nc.tensor.matmul(v_ps[:], wkv_sb[:, c, HD: 2 * HD], x_c,
                                 start=st, stop=sp, skip_group_check=True)
                for h in range(NH):
                    nc.tensor.matmul(q_ps[h][:], wq_sb[:, c, h * HD:(h + 1) * HD],
                                     x_c, start=st, stop=sp, skip_group_check=True)

            # PSUM -> SBUF bf16 staging for rope + AV
            nc.scalar.copy(qsb[0][:, 0, :], q_ps[0][:])
            nc.scalar.copy(qsb[0][:, 1, :], q_ps[1][:])
            nc.vector.tensor_scalar_mul(ksb[:], k_ps[:], 1.0)
            vnT_sb = const.tile([128, TOK], bf, name="vnT")
            nc.vector.tensor_scalar_mul(vnT_sb[:], v_ps[:], 1.0)
            nc.scalar.copy(qsb[1][:, 0, :], q_ps[2][:])
            nc.scalar.copy(qsb[1][:, 1, :], q_ps[3][:])

        # ---- phase 1b: RoPE via PE half-swap ----
        # qT = q * [c;c] + swap(q) * [-s;s]; swap(q) comes from one matmul
        # against the host-fed half-swap permutation.
        rtmp = ctx.enter_context(tc.tile_pool(name="rope_tmp", bufs=4))
        with tc.tile_pool(name="rope_ps", bufs=1, space="PSUM") as rope_pool, \
             tc.tile_pool(name="vt_ps", bufs=2, space="PSUM") as vt_pool:
            # keep the PE hot through the Act-copy latency
            warm2 = rope_pool.tile([128, 2, TOK], f32, tag="qsw", name="warm2")
            nc.tensor.matmul(
                warm2[:].rearrange("p h t -> p (h t)"), warm_sb[:, 0:128],
                warm_sb[:], start=True, stop=True, skip_group_check=True,
            )

            def rope_pair(src_sb, dst, wide):
                """src_sb/dst: [128, 2, TOK] (pair) or [128, TOK] (k)."""
                hshape = [128, 2, TOK] if wide else [128, TOK]
                sw_ps = rope_pool.tile([128, 2, TOK], f32, tag="qsw", name="qsw")
                sw = sw_ps[:] if wide else sw_ps[:, 0, :]
                flat_in = src_sb.rearrange("p h t -> p (h t)") if wide else src_sb
                flat_sw = sw.rearrange("p h t -> p (h t)") if wide else sw
                nc.tensor.matmul(flat_sw, swp_sb[:], flat_in,
                                 start=True, stop=True, skip_group_check=True)
                cc = cc2_sb[:].unsqueeze(1).broadcast_to((128, 2, TOK)) if wide else cc2_sb[:]
                ss = ss2_sb[:].unsqueeze(1).broadcast_to((128, 2, TOK)) if wide else ss2_sb[:]
                t1 = rtmp.tile(hshape, bf, tag="rt1", name="rt1")
                t2 = rtmp.tile(hshape, bf, tag="rt2", name="rt2")
                nc.vector.tensor_mul(t1[:], src_sb, cc)
                nc.vector.tensor_mul(t2[:], sw, ss)
                nc.vector.tensor_add(dst, t1[:], t2[:])

            rope_pair(qsb[0][:, :, :], qT_sb[0][:, :, :], True)
            # V_new^T -> per-batch V_new [t=32, hd]: PE transposes fill the
            # PE while the DVE finishes pair 0's rope.
            for b in range(B):
                vt = vt_pool.tile([S, HD], bf, tag="vt", name="vt")
                nc.tensor.transpose(vt[:], vnT_sb[:, b * S:(b + 1) * S], ident_sb[:])
                nc.scalar.copy(vn_sb[:, b, :], vt[:])
            rope_pair(ksb[:], kTn_sb[:], False)
            rope_pair(qsb[1][:, :, :], qT_sb[1][:, :, :], True)

        # ---- phase 2+3: attention in two head-pair passes ----
        with tc.tile_pool(name="s_ps", bufs=3, space="PSUM") as s_pool, \
             tc.tile_pool(name="acc_ps", bufs=1, space="PSUM") as acc_pool, \
             tc.tile_pool(name="attn", bufs=6) as attn_pool, \
             tc.tile_pool(name="pair", bufs=4) as pair_pool:
            for p in range(NHP):
                qpair = qT_sb[p][:, :, :]  # [128, 2, TOK]
                o_ps = acc_pool.tile([128, 2, TOK], f32, tag="o", name="o")
                sum_ps = acc_pool.tile([1, 2 * TOK], f32, tag="sum", name="sum")
                nc.vector.memset(sacc_sb[:], 0.0)

                # new-token block first (independent of the cache loop) so
                # the normalize/exchange chain at the pass end only waits on
                # the last cache group.
                sn_ps = s_pool.tile([S, B, 2, S], f32, tag="s", name="sn")
                anp = attn_new[p][:, :, :, :]  # [S, 2, B, S]
                for b in range(B):
                    nc.tensor.matmul(
                        sn_ps[0:S, b, :, :].rearrange("p h s -> p (h s)"),
                        kTn_sb[:, b * S:(b + 1) * S],
                        qpair[:, :, b * S:(b + 1) * S], start=True, stop=True,
                        skip_group_check=True,
                    )
                nc.vector.tensor_tensor(
                    out=sn_ps[:, :, :, :],
                    in0=sn_ps[:, :, :, :],
                    in1=maskt_sb[:].unsqueeze(2).broadcast_to((S, B, 2, S)),
                    op=mybir.AluOpType.add,
                )
                nc.scalar.activation(
                    anp.rearrange("p h b s -> p b h s"), sn_ps[:, :, :, :], EXP
                )
                nc.tensor.matmul(
                    sum_ps[0:1, :], ones_sb[0:S, 0:1],
                    anp.rearrange("p h b s -> p (h b s)"),
                    start=True, stop=False, skip_group_check=True,
                )
                for b in range(B):
                    for l in range(2):
                        nc.tensor.matmul(
                            o_ps[:, l, b * S:(b + 1) * S], vn_sb[:, b, :],
                            anp[:, l, b, :],
                            start=(b == 0 and l == 0), stop=False,
                            skip_group_check=True,
                        )

                # cache loop: 2-chunk groups; PE does scores+AV; DVE
                # accumulates the exp tiles for the deferred row sums with a
                # bf16 pair/quad tree to cut SBUF traffic (the f32 carry only
                # absorbs one add per two groups). The last group's sums go
                # through PE ones-matmuls instead so the pass tail does not
                # wait on the DVE accumulation.
                work = []
                held = []  # pending even-group pair sum

                def drain_one():
                    a2, g = work.pop(0)
                    for u in range(2):
                        t = 2 * g + u
                        pn = 128 if t < n_tc - 1 else tail
                        nc.tensor.matmul(
                            o_ps[:, :, :], vc_sb[0:pn, t, :], a2[0:pn, u, :, :],
                            start=False, stop=(t == n_tc - 1),
                            skip_group_check=True,
                        )
                        if g == n_tg - 1:
                            nc.tensor.matmul(
                                sum_ps[0:1, :], ones_sb[0:pn, 0:1],
                                a2[0:pn, u, :, :].rearrange("p h t -> p (h t)"),
                                start=False, stop=False, skip_group_check=True,
                            )
                    if g < n_tg - 1:
                        tg = pair_pool.tile([128, 2, TOK], bf, tag="tg", name="tg")
                        nc.vector.tensor_add(tg[:], a2[:, 0, :, :], a2[:, 1, :, :])
                        if g == n_tg - 2:
                            nc.vector.tensor_add(sacc_sb[:], sacc_sb[:], tg[:])
                        elif held:
                            tp = held.pop()
                            tq = pair_pool.tile([128, 2, TOK], bf, tag="tq", name="tq")
                            nc.vector.tensor_add(tq[:], tp[:], tg[:])
                            nc.vector.tensor_add(sacc_sb[:], sacc_sb[:], tq[:])
                        else:
                            held.append(tg)
                    if g == n_tg - 2:
                        # convert the carry while the last group computes
                        nc.vector.tensor_scalar_mul(saccb_sb[:], sacc_sb[:], 1.0)

                for g in range(n_tg):
                    s2 = s_pool.tile([128, 2, 2, TOK], f32, tag="s", name="s")
                    for u in range(2):
                        t = 2 * g + u
                        pn = 128 if t < n_tc - 1 else tail
                        nc.tensor.matmul(
                            s2[0:pn, u, :, :], kct_sb[:, t * 128: t * 128 + pn],
                            qpair, start=True, stop=True, skip_group_check=True,
                        )
                    a2 = attn_pool.tile([128, 2, 2, TOK], bf, tag="a", name="a")
                    nc.scalar.activation(a2[:, :, :, :], s2[:, :, :, :], EXP)
                    work.append((a2, g))
                    if len(work) > 1:
                        drain_one()
                while work:
                    drain_one()

                # fold the DVE carry in via one matmul and close the sums
                nc.tensor.matmul(
                    sum_ps[0:1, :], ones_sb[0:128, 0:1],
                    saccb_sb[:].rearrange("p h t -> p (h t)"),
                    start=False, stop=True, skip_group_check=True,
                )

                # 1/rowsum -> broadcast -> normalize on PSUM->SBUF copy
                nc.vector.reciprocal_approx_fast(recip_sb[p][:], sum_ps[0:1, :])
                nc.vector.tensor_scalar_mul(recipb_sb[p][:], recip_sb[p][:], 1.0)
                nc.gpsimd.partition_broadcast(recip_bc[p][:], recipb_sb[p][:])
                nc.vector.tensor_mul(
                    attnout[p][:], o_ps[:, :, :].rearrange("p h t -> p (h t)"),
                    recip_bc[p][:],
                )

                # all-gather this pass's attention outputs. The ag_in fill
                # rides the sync queue and the readback pieces ride the
                # scalar queue: a shared queue would head-of-line-block
                # pass 1's fill behind pass 0's readback (which waits on
                # the collective), serializing the exchanges.
                nc.sync.dma_start(ag_in[p][:], attnout[p][:])
                nc.gpsimd.collective_compute(
                    "AllGather", mybir.AluOpType.bypass,
                    replica_groups=[list(range(cores))],
                    ins=[ag_in[p].opt()], outs=[ag_out[p].opt()],
                )
                # per-source readback pieces: the Wo block for source j
                # unblocks as soon as its piece lands. Pass 1's pieces ride
                # the (then-idle) sync queue, emitted after ag_in[1] so they
                # never block it.
                ag_r = ag_out[p].rearrange("(r p) n -> p r n", p=128)
                for j in range(cores):
                    eng = nc.scalar if p == 0 else nc.sync
                    eng.dma_start(allx[p][:, j, :], ag_r[:, j, :])

        # ---- phase 4: out = attnout_all @ Wo[:, slice], per pass ----
        # Wo row blocks are ordered (j, p, l) = head 4j + 2p + l on the
        # host. Pass-0's half runs as soon as its exchange lands (while
        # pass-1 attention streams); pass-1's half waits only on its own
        # exchange.
        with tc.tile_pool(name="wo_ps", bufs=1, space="PSUM") as wo_pool:
            out_ps = [wo_pool.tile([128, outc], f32, tag=f"out{k}", name=f"out{k}")
                      for k in range(2)]
            out_r = out_d.ap().rearrange("(k p) n -> p k n", p=128)
            for p in range(NHP):
                for k in range(2):
                    for j in range(cores):
                        for l in range(2):
                            g = j * NH + 2 * p + l
                            nc.tensor.matmul(
                                out_ps[k][:],
                                allx[p][:, j, l * TOK + k * 128: l * TOK + k * 128 + 128],
                                wo_sb[:, g, :],
                                start=(p == 0 and j == 0 and l == 0),
                                stop=(p == NHP - 1 and j == cores - 1 and l == 1),
                                skip_group_check=True,
                            )
                    if p == NHP - 1:
                        nc.scalar.copy(out_sb[:, k, :], out_ps[k][:])
                        nc.sync.dma_start(out_r[:, k, :], out_sb[:, k, :])

    nc.compile()
    return nc


def _pack(a):
    return np.ascontiguousarray(a)


def prep_in_maps(x, freqs_cos, freqs_sin, mask, cache_k, cache_v, Wq, Wk, Wv, Wo,
                 cores=CORES, gather="cc"):
    """Host-side sharding + partition-major packing."""
    n_dc = D // 128
    n_tc = (PREV + 127) // 128
    outc = D // cores

    x = np.asarray(x, np.float32).reshape(TOK, D)
    xt = x.T.astype(BF16)  # [D, TOK]
    xt_p = _pack(xt.reshape(n_dc, 128, TOK).transpose(1, 0, 2))  # [128, n_dc, TOK]
    cost = np.tile(np.asarray(freqs_cos, np.float32)[0].T, (1, B))  # [64, TOK]
    sint = np.tile(np.asarray(freqs_sin, np.float32)[0].T, (1, B))
    cc2 = _pack(np.concatenate([cost, cost], axis=0).astype(BF16))  # [128, TOK]
    ss2 = _pack(np.concatenate([-sint, sint], axis=0).astype(BF16))
    swp = np.zeros((128, 128), np.float32)
    swp[(np.arange(128) + 64) % 128, np.arange(128)] = 1.0
    swp = _pack(swp.astype(BF16))
    maskt = _pack(np.asarray(mask, np.float32).transpose(2, 0, 1).reshape(S, TOK))
    Wq = np.asarray(Wq, np.float32)
    Wk = np.asarray(Wk, np.float32)
    Wv = np.asarray(Wv, np.float32)
    Wo = np.asarray(Wo, np.float32)
    cache_k = np.asarray(cache_k, np.float32)
    cache_v = np.asarray(cache_v, np.float32)

    in_maps = []
    for c in range(cores):
        wq_c = (Wq[:, c * QCOLS:(c + 1) * QCOLS] * SCALE).reshape(D, NH, HD)[
            :, :, _IDX
        ].reshape(D, QCOLS).astype(BF16)
        wq_p = _pack(wq_c.reshape(n_dc, 128, QCOLS).transpose(1, 0, 2))
        wk_c = Wk[:, c * HD:(c + 1) * HD][:, _IDX]
        wv_c = Wv[:, c * HD:(c + 1) * HD]
        wkv_c = np.concatenate([wk_c, wv_c], axis=1).astype(BF16)
        wkv_p = _pack(wkv_c.reshape(n_dc, 128, 2 * HD).transpose(1, 0, 2))
        kct_c = _pack(cache_k[0, :PREV, c, :][:, _IDX].T.astype(BF16))  # [HD, PREV]
        vc_full = np.zeros((n_tc * 128, HD), np.float32)
        vc_full[:PREV] = cache_v[0, :PREV, c, :]
        vc_p = _pack(vc_full.astype(BF16).reshape(n_tc, 128, HD).transpose(1, 0, 2))
        # Wo rows ordered: block (j, p, l) = head 4j + 2p + l
        wo_c = Wo[:, c * outc:(c + 1) * outc].astype(BF16)  # [H*HD, outc]
        wo_blocks = wo_c.reshape(H, HD, outc)
        order = [4 * j + 2 * p + l
                 for j in range(cores) for p in range(NHP) for l in range(2)]
        wo_x = wo_blocks[order]  # [32, HD, outc]
        wo_p = _pack(wo_x.transpose(1, 0, 2))  # [128, H, outc]
        in_maps.append(
            {
                "xt": xt_p,
                "wq": wq_p,
                "wkv": wkv_p,
                "kct": kct_c,
                "vc": vc_p,
                "wo": wo_p,
                "cc2": cc2,
                "ss2": ss2,
                "swp": swp,
                "maskt": maskt,
            }
        )
    return in_maps


def kernel(x, freqs_cos, freqs_sin, mask, cache_k, cache_v, Wq, Wk, Wv, Wo, positions):
    global LAST_EXEC_NS, LAST_RES
    assert int(positions) == PREV, f"kernel compiled for positions={PREV}"

    key = ("v3", os.environ.get("KERNEL_GATHER", "cc"))
    if key not in _BUILD_CACHE:
        _BUILD_CACHE[key] = build(CORES, key[1])
    nc = _BUILD_CACHE[key]

    in_maps = prep_in_maps(
        x, freqs_cos, freqs_sin, mask, cache_k, cache_v, Wq, Wk, Wv, Wo,
        CORES, key[1]
    )

    trace = os.environ.get("KERNEL_TRACE", "0") == "1"
    if trace:
        _install_ntff_hook()
    res = run_bass_kernel_spmd(
        nc, in_maps, core_ids=list(range(CORES)), trace=trace
    )
    if trace:
        LAST_EXEC_NS = res.exec_time_ns
        LAST_RES = res

    outc = D // CORES
    out = np.empty((TOK, D), np.float32)
    for c in range(CORES):
        out[:, c * outc:(c + 1) * outc] = res.results[c]["out"]
    return out.reshape(B, S, D)
